# revision 1
# baseline (speedup 1.0000x reference)
"""Trainium2 Bass kernel: temporal-masked MHA + top2-gated MoE layer (8 NeuronCores).

Strategy:
  - data-parallel attention over B (8 batches -> 8 cores), computed in
    transposed layout (feature dim on partitions) with fp32r matmuls
  - AllToAll to regroup tokens by sequence position for the MoE
    (groups = L positions, 8 tokens each = the batch entries)
  - on-chip top-2 routing w/ capacity, dispatch/combine via indirect-DMA
    row gathers, expert FFN streamed from HBM in fp32r
"""

import math
from contextlib import ExitStack

import numpy as np

import concourse.bass as bass
import concourse.bacc as bacc
import concourse.mybir as mybir
import concourse.tile as tile
from concourse.bass_utils import run_bass_kernel_spmd
from concourse.masks import make_identity

F32 = mybir.dt.float32
F32R = mybir.dt.float32r
F16 = mybir.dt.float16
I32 = mybir.dt.int32
AX = mybir.AxisListType
OP = mybir.AluOpType
ACT = mybir.ActivationFunctionType
P = 128

FULL = dict(L=512, B=8, E=1024, H=16, HID=4096, NE=5, NC=8)


def make_cfg(d):
    c = dict(d)
    c["CAP"] = max(min(c["B"], int(c["B"] * 2.0 / c["NE"])), 4)
    c["D"] = c["E"] // c["H"]
    assert c["D"] == 64, "head dim assumed 64"
    assert c["B"] == c["NC"]
    c["LC"] = c["L"] // c["NC"]          # L-groups per core
    c["TOK"] = c["LC"] * c["B"]          # MoE tokens per core
    assert c["TOK"] % P == 0
    assert c["L"] % P == 0
    c["ROW"] = c["E"] + 8                # A2A row: E cols of y + NE logits + pad
    assert c["NE"] <= 8
    c["GCAP"] = c["LC"] * c["CAP"]       # slots per expert per core
    return c


def tl(pool, shape, dtype=F32, *, tag, bufs=None):
    return pool.tile(list(shape), dtype, tag=tag, name=tag, bufs=bufs)


def build_bass(c):
    nc = bacc.Bacc("TRN2", target_bir_lowering=False, debug=False,
                   num_devices=c["NC"])
    L, B, E, H, HID, NE = c["L"], c["B"], c["E"], c["H"], c["HID"], c["NE"]
    CAP, LC, TOK, ROW, GCAP = c["CAP"], c["LC"], c["TOK"], c["ROW"], c["GCAP"]
    KT = E // P                       # k-tiles over E
    MTOK = L // P                     # token tiles (attention, per batch)
    NTOKT = TOK // P                  # token tiles (MoE)
    NHT = 2 * E // P                  # qk row tiles
    HIDT = HID // P
    GPT = P // B                      # groups per 128-token tile
    NCH = min(512, E)                 # matmul N-chunk over E
    ECH = E // NCH
    HPC = NCH // 64                   # heads per N-chunk
    spt = min(P, GCAP)                # slots per slot-tile
    nslt = (GCAP + P - 1) // P        # slot tiles per expert
    tpst = spt // (GPT * CAP)         # token-tiles per slot-tile
    G1 = min(4, HIDT)                 # w1 N-group (psum tiles per weight load)
    G2 = min(4, KT)                   # w2 N-group

    # ---- I/O ----
    dt_ = nc.dram_tensor
    xT = dt_("xT", [E, L], F32, kind="ExternalInput")[:]
    tcol = dt_("tcol", [L, 1], F32, kind="ExternalInput")[:]
    trep = dt_("trep", [P, L], F32, kind="ExternalInput")[:]
    wqkvT = dt_("wqkvT", [E, 3 * E], F32, kind="ExternalInput")[:]
    bqk = dt_("bqk", [2 * E, 1], F32, kind="ExternalInput")[:]
    bvrep = dt_("bvrep", [P, E], F32, kind="ExternalInput")[:]
    woutT = dt_("woutT", [E, E], F32, kind="ExternalInput")[:]
    bout = dt_("bout", [E, 1], F32, kind="ExternalInput")[:]
    ln1g = dt_("ln1g", [E, 1], F32, kind="ExternalInput")[:]
    ln1b = dt_("ln1b", [E, 1], F32, kind="ExternalInput")[:]
    ln2g = dt_("ln2grep", [P, E], F32, kind="ExternalInput")[:]
    ln2b = dt_("ln2brep", [P, E], F32, kind="ExternalInput")[:]
    gw = dt_("gatew", [E, NE], F32, kind="ExternalInput")[:]
    w1 = dt_("w1", [NE, E, HID], F16, kind="ExternalInput")[:]
    w2 = dt_("w2", [NE, HID, E], F16, kind="ExternalInput")[:]
    out = dt_("out", [TOK, E], F32, kind="ExternalOutput")[:]

    # ---- host-side constant tables (baked into the NEFF) ----
    tri = np.zeros((P, P), np.float32)       # strict-lower within B-groups
    ob = np.zeros((P, P), np.float32)        # all-ones within B-groups
    for i in range(P):
        for j in range(P):
            if i // B == j // B:
                ob[i, j] = 1.0
                if i < j:
                    tri[i, j] = 1.0
    nsel = np.zeros((P, GPT), np.float32)
    for i in range(P):
        nsel[i, i // B] = float(i % B)
    iotac = np.tile(np.arange(CAP, dtype=np.float32), (P, 1))
    iotae = np.tile(np.arange(NE, dtype=np.float32), (P, 1))
    gbase = np.zeros((nslt, spt, 1), np.float32)
    for st in range(nslt):
        for p in range(spt):
            gbase[st, p, 0] = float(B * ((st * P + p) // CAP))
    gb2 = np.zeros((NTOKT, P, 1), np.float32)
    for t in range(NTOKT):
        for p in range(P):
            gb2[t, p, 0] = float(CAP * ((t * P + p) // B))

    with tile.TileContext(nc) as tc, ExitStack() as ctx:
        cst = ctx.enter_context(tc.tile_pool(name="cst", bufs=1))
        dram = ctx.enter_context(tc.tile_pool(name="dram", bufs=1, space="DRAM"))
        pB = ctx.enter_context(tc.tile_pool(name="pB", bufs=1))

        def const_tile(arr, tag):
            ap = nc.inline_tensor(np.ascontiguousarray(arr), name=tag)[:]
            t = tl(cst, list(arr.shape), F32, tag=tag)
            nc.gpsimd.dma_start(t[:], ap)
            return t

        ident = tl(cst, [P, P], F32, tag="ident")
        make_identity(nc, ident[:])
        ident16 = tl(cst, [P, P], F16, tag="ident16")
        make_identity(nc, ident16[:])
        ones_t = tl(cst, [P, 1], F32, tag="ones")
        nc.vector.memset(ones_t[:], 1.0)
        onesr_t = tl(cst, [1, P], F32, tag="onesr")
        nc.vector.memset(onesr_t[:], 1.0)
        tri_t = const_tile(tri, "tri")
        ob_t = const_tile(ob, "ob")
        nsel_t = const_tile(nsel, "nsel")
        iotac_t = const_tile(iotac, "iotac")
        iotae_t = const_tile(iotae, "iotae")
        gbase_ap = nc.inline_tensor(gbase, name="gbase")[:]
        gbase_ts = []
        for st in range(nslt):
            t = tl(cst, [spt, 1], F32, tag=f"gbase{st}")
            nc.gpsimd.dma_start(t[:], gbase_ap[st])
            gbase_ts.append(t)
        gb2_ap = nc.inline_tensor(gb2, name="gb2")[:]
        gb2_ts = []
        for tt in range(NTOKT):
            t = tl(cst, [P, 1], F32, tag=f"gb2{tt}")
            nc.gpsimd.dma_start(t[:], gb2_ap[tt])
            gb2_ts.append(t)

        send = tl(dram, [L, ROW], F32, tag="send")

        # =========================================================
        # PHASE A: attention for this core's batch (all in SBUF-T layout)
        # =========================================================
        with tc.tile_pool(name="pA", bufs=1) as pA:
            xt = []
            for k in range(KT):
                t = tl(pA, [P, L], F32, tag=f"xt{k}")
                nc.gpsimd.dma_start(t[:], xT[k * P:(k + 1) * P, :])
                xt.append(t)
            bqk_t = tl(pA, [P, NHT], F32, tag="bqk")
            nc.gpsimd.dma_start(bqk_t[:], bqk.rearrange("(m p) o -> p (m o)", p=P))
            bv_t = []
            for nn in range(ECH):
                t = tl(pA, [P, NCH], F32, tag=f"bv{nn}")
                nc.gpsimd.dma_start(t[:], bvrep[:, nn * NCH:(nn + 1) * NCH])
                bv_t.append(t)
            tcol_t = tl(pA, [P, MTOK], F32, tag="tcol")
            nc.gpsimd.dma_start(tcol_t[:], tcol.rearrange("(m p) o -> p (m o)", p=P))
            trep_t = tl(pA, [P, L], F32, tag="trep")
            nc.gpsimd.dma_start(trep_t[:], trep)
            gw_t = tl(pA, [P, KT * NE], F32, tag="gw")
            nc.sync.dma_start(gw_t[:].rearrange("p (k e) -> p k e", e=NE),
                              gw.rearrange("(k p) e -> p k e", p=P))
            bout_t = tl(pA, [P, KT], F32, tag="bout")
            nc.gpsimd.dma_start(bout_t[:], bout.rearrange("(m p) o -> p (m o)", p=P))
            ln1g_t = tl(pA, [P, KT], F32, tag="ln1g")
            nc.gpsimd.dma_start(ln1g_t[:], ln1g.rearrange("(m p) o -> p (m o)", p=P))
            ln1b_t = tl(pA, [P, KT], F32, tag="ln1b")
            nc.gpsimd.dma_start(ln1b_t[:], ln1b.rearrange("(m p) o -> p (m o)", p=P))

            # ---- qkT = wqkv[:2E] @ x^T ; V token-major with per-head ones col
            qk = [tl(pA, [P, L], F32, tag=f"qk{m}") for m in range(NHT)]
            VW = H * 65
            vt = [tl(pA, [P, VW], F32, tag=f"vt{m}") for m in range(MTOK)]
            with tc.tile_pool(name="wp", bufs=4) as wp, \
                 tc.tile_pool(name="psQ", bufs=2, space="PSUM") as psQ:
                for m in range(NHT):
                    ps = tl(psQ, [P, L], F32, tag="ps")
                    for k in range(KT):
                        lw = tl(wp, [P, P], F32, tag="lw")
                        nc.sync.dma_start(
                            lw[:], wqkvT[k * P:(k + 1) * P, m * P:(m + 1) * P])
                        nc.tensor.matmul(ps[:], lhsT=lw[:], rhs=xt[k][:],
                                         start=(k == 0), stop=(k == KT - 1))
                    nc.vector.tensor_scalar_add(qk[m][:], ps[:], bqk_t[:, m:m + 1])
                for mt in range(MTOK):
                    for nn in range(ECH):
                        ps = tl(psQ, [P, NCH], F32, tag="ps")
                        for k in range(KT):
                            rw = tl(wp, [P, NCH], F32, tag="rw")
                            nc.sync.dma_start(
                                rw[:], wqkvT[k * P:(k + 1) * P,
                                             2 * E + nn * NCH:2 * E + (nn + 1) * NCH])
                            nc.tensor.matmul(
                                ps[:], lhsT=xt[k][:, mt * P:(mt + 1) * P],
                                rhs=rw[:], start=(k == 0), stop=(k == KT - 1))
                        dst = vt[mt][:].rearrange("p (h e) -> p h e", e=65)[
                            :, nn * HPC:(nn + 1) * HPC, 0:64]
                        nc.vector.tensor_add(
                            dst, ps[:].rearrange("p (h e) -> p h e", e=64),
                            bv_t[nn][:].rearrange("p (h e) -> p h e", e=64))
                    nc.vector.memset(
                        vt[mt][:].rearrange("p (h e) -> p h e", e=65)[:, :, 64:65],
                        1.0)

            # ---- temporal masks (0 / -8e9; exp scale 1/8 folds to -1e9)
            maskb = [tl(pA, [P, L], F32, tag=f"mb{kt}") for kt in range(MTOK)]
            for kt in range(MTOK):
                nc.vector.tensor_tensor(
                    out=maskb[kt][:],
                    in0=tcol_t[:, kt:kt + 1].to_broadcast([P, L]),
                    in1=trep_t[:], op=OP.is_gt)
                nc.vector.tensor_scalar_mul(maskb[kt][:], maskb[kt][:],
                                            -1e9 / (1.0 / math.sqrt(64)))

            # ---- heads
            attnT = [tl(pA, [P, L], F32, tag=f"at{k}") for k in range(KT)]
            sc = 1.0 / math.sqrt(64)
            with tc.tile_pool(name="pp", bufs=2) as pp, \
                 tc.tile_pool(name="smp", bufs=3) as smp, \
                 tc.tile_pool(name="psS", bufs=4, space="PSUM") as psS, \
                 tc.tile_pool(name="psAV", bufs=2, space="PSUM") as psAV:
                for h in range(H):
                    mq, rq = (h * 64) // P, (h * 64) % P
                    mk, rk = (E + h * 64) // P, (E + h * 64) % P
                    qT = qk[mq][rq:rq + 64, :]
                    av = tl(psAV, [65, L], F32, tag="av")
                    pts = []
                    for kt in range(MTOK):
                        sps = tl(psS, [P, L], F32, tag="sps")
                        nc.tensor.matmul(
                            sps[:],
                            lhsT=qk[mk][rk:rk + 64, kt * P:(kt + 1) * P],
                            rhs=qT, start=True, stop=True)
                        tmp = tl(pp, [P, L], F32, tag="tmp")
                        nc.vector.tensor_add(tmp[:], sps[:], maskb[kt][:])
                        pt_ = tl(pp, [P, L], F32, tag=f"pt{kt}")
                        nc.scalar.activation(pt_[:], tmp[:], ACT.Exp, scale=sc)
                        pts.append(pt_)
                    for kt in range(MTOK):
                        nc.tensor.matmul(
                            av[:], lhsT=vt[kt][:, h * 65:h * 65 + 65],
                            rhs=pts[kt][:], start=(kt == 0),
                            stop=(kt == MTOK - 1))
                    rec = tl(smp, [1, L], F32, tag="rec")
                    nc.vector.reciprocal(rec[:], av[64:65, :])
                    rep_ps = tl(psS, [P, L], F32, tag="repps", bufs=2)
                    nc.tensor.matmul(rep_ps[:], lhsT=onesr_t[:], rhs=rec[:],
                                     start=True, stop=True)
                    rep = tl(smp, [P, L], F32, tag="rep")
                    nc.vector.tensor_copy(rep[:], rep_ps[:])
                    nc.vector.tensor_mul(attnT[mq][rq:rq + 64, :], av[0:64, :],
                                         rep[0:64, :])

            # ---- out-proj + residual (into xt) ; LN1 -> yT (reuses qk slots)
            with tc.tile_pool(name="wp2", bufs=4) as wp2, \
                 tc.tile_pool(name="psO", bufs=2, space="PSUM") as psO:
                for m in range(KT):
                    ps = tl(psO, [P, L], F32, tag="ps")
                    for k in range(KT):
                        lw = tl(wp2, [P, P], F32, tag="lw")
                        nc.sync.dma_start(
                            lw[:], woutT[k * P:(k + 1) * P, m * P:(m + 1) * P])
                        nc.tensor.matmul(ps[:], lhsT=lw[:],
                                         rhs=attnT[k][:],
                                         start=(k == 0), stop=(k == KT - 1))
                    nc.vector.tensor_scalar_add(ps[:], ps[:], bout_t[:, m:m + 1])
                    nc.vector.tensor_add(xt[m][:], ps[:], xt[m][:])  # zT

            yT = [tl(pA, [P, L], F32, tag=f"qk{k}") for k in range(KT)]
            with tc.tile_pool(name="lnp", bufs=3) as lnp, \
                 tc.tile_pool(name="psL", bufs=1, space="PSUM") as psL:
                mu_ps = tl(psL, [1, L], F32, tag="mu")
                sq_ps = tl(psL, [1, L], F32, tag="sq")
                for k in range(KT):
                    nc.tensor.matmul(mu_ps[:], lhsT=ones_t[:],
                                     rhs=xt[k][:],
                                     start=(k == 0), stop=(k == KT - 1))
                for k in range(KT):
                    sqt = tl(lnp, [P, L], F32, tag="sqt")
                    nc.scalar.activation(sqt[:], xt[k][:], ACT.Square)
                    nc.tensor.matmul(sq_ps[:], lhsT=ones_t[:], rhs=sqt[:],
                                     start=(k == 0), stop=(k == KT - 1))
                mu_r = tl(lnp, [1, L], F32, tag="mu_r")
                nc.vector.tensor_scalar_mul(mu_r[:], mu_ps[:], 1.0 / E)
                var_r = tl(lnp, [1, L], F32, tag="var_r")
                nc.vector.tensor_scalar_mul(var_r[:], sq_ps[:], 1.0 / E)
                mu2 = tl(lnp, [1, L], F32, tag="mu2")
                nc.vector.tensor_mul(mu2[:], mu_r[:], mu_r[:])
                nc.vector.tensor_sub(var_r[:], var_r[:], mu2[:])
                nc.vector.tensor_scalar_add(var_r[:], var_r[:], 1e-5)
                nc.scalar.sqrt(var_r[:], var_r[:])
                rstd_r = tl(lnp, [1, L], F32, tag="rstd_r")
                nc.vector.reciprocal(rstd_r[:], var_r[:])
                murep_ps = tl(psL, [P, L], F32, tag="murep")
                nc.tensor.matmul(murep_ps[:], lhsT=onesr_t[:], rhs=mu_r[:],
                                 start=True, stop=True)
                mu_rep = tl(lnp, [P, L], F32, tag="mu_rep")
                nc.vector.tensor_copy(mu_rep[:], murep_ps[:])
                rsrep_ps = tl(psL, [P, L], F32, tag="rsrep")
                nc.tensor.matmul(rsrep_ps[:], lhsT=onesr_t[:], rhs=rstd_r[:],
                                 start=True, stop=True)
                rstd_rep = tl(lnp, [P, L], F32, tag="rstd_rep")
                nc.vector.tensor_copy(rstd_rep[:], rsrep_ps[:])
                for k in range(KT):
                    t1 = tl(lnp, [P, L], F32, tag="t1")
                    nc.vector.tensor_sub(t1[:], xt[k][:], mu_rep[:])
                    nc.vector.tensor_mul(t1[:], t1[:], rstd_rep[:])
                    nc.vector.tensor_scalar(
                        out=yT[k][:], in0=t1[:], scalar1=ln1g_t[:, k:k + 1],
                        scalar2=ln1b_t[:, k:k + 1], op0=OP.mult, op1=OP.add)

            # ---- gate logits; transpose everything token-major -> send
            with tc.tile_pool(name="gp", bufs=3) as gp, \
                 tc.tile_pool(name="psG", bufs=2, space="PSUM") as psG:
                gps = tl(psG, [NE, L], F32, tag="gps")
                for k in range(KT):
                    nc.tensor.matmul(
                        gps[:], lhsT=gw_t[:, k * NE:(k + 1) * NE],
                        rhs=yT[k][:], start=(k == 0), stop=(k == KT - 1))
                lg_sb = tl(gp, [NE, L], F32, tag="lg_sb")
                nc.vector.tensor_copy(lg_sb[:], gps[:])
                for ct in range(MTOK):
                    tp2 = tl(psG, [P, NE], F32, tag="tp2")
                    nc.tensor.transpose(tp2[:, 0:NE],
                                        lg_sb[:, ct * P:(ct + 1) * P],
                                        ident[0:NE, 0:NE])
                    lgr = tl(gp, [P, 8], F32, tag="lgr")
                    nc.vector.memset(lgr[:, NE:8], 0.0)
                    nc.vector.tensor_copy(lgr[:, 0:NE], tp2[:, 0:NE])
                    nc.sync.dma_start(send[ct * P:(ct + 1) * P, E:E + 8], lgr[:])
                    yrow = tl(gp, [P, E], F32, tag="yrow")
                    for k in range(KT):
                        tp = tl(psG, [P, P], F32, tag="tp")
                        nc.tensor.transpose(tp[:], yT[k][:, ct * P:(ct + 1) * P],
                                            ident[:])
                        nc.vector.tensor_copy(yrow[:, k * P:(k + 1) * P], tp[:])
                    nc.sync.dma_start(send[ct * P:(ct + 1) * P, 0:E], yrow[:])

        # =========================================================
        # AllToAll + permute to group-major token order
        # =========================================================
        recv = tl(dram, [L, ROW], F32, tag="recv")
        nc.gpsimd.collective_compute(
            "AllToAll", OP.bypass,
            replica_groups=[list(range(c["NC"]))],
            ins=[send[:].opt()], outs=[recv[:].opt()])
        ybuf = tl(dram, [TOK, E], F32, tag="ybuf")
        nc.sync.dma_start(
            ybuf[:].rearrange("(l i) r -> l i r", i=c["NC"]),
            recv[:][:, 0:E].rearrange("(i l) r -> l i r", i=c["NC"]))
        lgbuf = tl(dram, [TOK, 8], F32, tag="lgbuf")
        nc.sync.dma_start(
            lgbuf[:].rearrange("(l i) r -> l i r", i=c["NC"]),
            recv[:][:, E:E + 8].rearrange("(i l) r -> l i r", i=c["NC"]))

        # =========================================================
        # PHASE B: top-2 routing with capacity
        # =========================================================
        nmat_d = tl(dram, [NTOKT, GPT, NE * CAP], F32, tag="nmat_d")
        gc1 = [None] * NTOKT
        gc2 = [None] * NTOKT
        idx1 = [None] * NTOKT
        idx2 = [None] * NTOKT
        with tc.tile_pool(name="rt", bufs=2) as rt, \
             tc.tile_pool(name="psR", bufs=2, space="PSUM") as psR:
            for tt in range(NTOKT):
                lg = tl(rt, [P, NE], F32, tag="lg")
                nc.gpsimd.dma_start(lg[:], lgbuf[tt * P:(tt + 1) * P, 0:NE])
                mx = tl(rt, [P, 1], F32, tag="mx")
                nc.vector.reduce_max(mx[:], lg[:], axis=AX.X)
                nc.vector.tensor_scalar_mul(mx[:], mx[:], -1.0)
                ex = tl(rt, [P, NE], F32, tag="ex")
                sm = tl(rt, [P, 1], F32, tag="sm")
                nc.scalar.activation(ex[:], lg[:], ACT.Exp, bias=mx[:],
                                     accum_out=sm[:])
                rcp = tl(rt, [P, 1], F32, tag="rcp")
                nc.vector.reciprocal(rcp[:], sm[:])
                raw = tl(rt, [P, NE], F32, tag="raw")
                nc.vector.tensor_scalar_mul(raw[:], ex[:], rcp[:])

                def top1(rawt, tag):
                    g = tl(rt, [P, 1], F32, tag=f"g{tag}")
                    nc.vector.reduce_max(g[:], rawt[:], axis=AX.X)
                    eq = tl(rt, [P, NE], F32, tag=f"eq{tag}")
                    nc.vector.tensor_tensor(
                        out=eq[:], in0=rawt[:],
                        in1=g[:].to_broadcast([P, NE]), op=OP.is_ge)
                    cs = tl(rt, [P, NE], F32, tag=f"cs{tag}")
                    nc.vector.memset(cs[:, 0:1], 0.0)
                    for j in range(1, NE):
                        nc.vector.tensor_add(cs[:, j:j + 1], cs[:, j - 1:j],
                                             eq[:, j - 1:j])
                    fst = tl(rt, [P, NE], F32, tag=f"fst{tag}")
                    nc.vector.tensor_scalar(out=fst[:], in0=cs[:], scalar1=0.5,
                                            scalar2=None, op0=OP.is_lt)
                    m_ = tl(rt, [P, NE], F32, tag=f"m{tag}")
                    nc.vector.tensor_mul(m_[:], eq[:], fst[:])
                    return g, m_

                g1, m1r = top1(raw, "1")
                raw2 = tl(rt, [P, NE], F32, tag="raw2")
                nc.vector.tensor_mul(raw2[:], raw[:], m1r[:])
                nc.vector.tensor_sub(raw2[:], raw[:], raw2[:])
                g2, m2r = top1(raw2, "2")
                den = tl(rt, [P, 1], F32, tag="den")
                nc.vector.tensor_add(den[:], g1[:], g2[:])
                nc.vector.tensor_scalar_add(den[:], den[:], 1e-9)
                rd = tl(rt, [P, 1], F32, tag="rd")
                nc.vector.reciprocal(rd[:], den[:])
                g1n = tl(rt, [P, 1], F32, tag="g1n")
                nc.vector.tensor_mul(g1n[:], g1[:], rd[:])
                g2n = tl(rt, [P, 1], F32, tag="g2n")
                nc.vector.tensor_mul(g2n[:], g2[:], rd[:])

                pos1 = tl(psR, [P, NE], F32, tag="pos1")
                nc.tensor.matmul(pos1[:], lhsT=tri_t[:], rhs=m1r[:],
                                 start=True, stop=True)
                keep1 = tl(rt, [P, NE], F32, tag="keep1")
                nc.vector.tensor_scalar(out=keep1[:], in0=pos1[:],
                                        scalar1=CAP - 0.5, scalar2=None,
                                        op0=OP.is_lt)
                m1 = tl(rt, [P, NE], F32, tag="m1")
                nc.vector.tensor_mul(m1[:], m1r[:], keep1[:])
                pos2 = tl(psR, [P, NE], F32, tag="pos2")
                nc.tensor.matmul(pos2[:], lhsT=tri_t[:], rhs=m2r[:],
                                 start=True, stop=False)
                nc.tensor.matmul(pos2[:], lhsT=ob_t[:], rhs=m1[:],
                                 start=False, stop=True)
                keep2 = tl(rt, [P, NE], F32, tag="keep2")
                nc.vector.tensor_scalar(out=keep2[:], in0=pos2[:],
                                        scalar1=CAP - 0.5, scalar2=None,
                                        op0=OP.is_lt)
                m2 = tl(rt, [P, NE], F32, tag="m2")
                nc.vector.tensor_mul(m2[:], m2r[:], keep2[:])

                def dotX(a_ap, b_ap, tag):
                    t5 = tl(rt, [P, NE], F32, tag=f"t5{tag}")
                    nc.vector.tensor_mul(t5[:], a_ap, b_ap)
                    o = tl(rt, [P, 1], F32, tag=f"o{tag}")
                    nc.vector.reduce_sum(o[:], t5[:], axis=AX.X)
                    return o

                m1f = tl(rt, [P, 1], F32, tag="m1f")
                nc.vector.reduce_sum(m1f[:], m1[:], axis=AX.X)
                m2f = tl(rt, [P, 1], F32, tag="m2f")
                nc.vector.reduce_sum(m2f[:], m2[:], axis=AX.X)
                gc1[tt] = tl(pB, [P, 1], F32, tag=f"gc1_{tt}")
                nc.vector.tensor_mul(gc1[tt][:], g1n[:], m1f[:])
                gc2[tt] = tl(pB, [P, 1], F32, tag=f"gc2_{tt}")
                nc.vector.tensor_mul(gc2[tt][:], g2n[:], m2f[:])
                p1 = dotX(pos1[:], m1[:], "p1")
                p2 = dotX(pos2[:], m2[:], "p2")
                e1 = dotX(iotae_t[:], m1[:], "e1")
                e2 = dotX(iotae_t[:], m2[:], "e2")

                for (en, pn, sl) in ((e1, p1, 1), (e2, p2, 2)):
                    f_ = tl(rt, [P, 1], F32, tag=f"if{sl}")
                    nc.vector.tensor_scalar(
                        out=f_[:], in0=en[:], scalar1=float(GCAP),
                        scalar2=gb2_ts[tt][:], op0=OP.mult, op1=OP.add)
                    nc.vector.tensor_add(f_[:], f_[:], pn[:])
                    ix = tl(pB, [P, 1], I32, tag=f"idx{sl}_{tt}")
                    nc.vector.tensor_copy(ix[:], f_[:])
                    (idx1 if sl == 1 else idx2)[tt] = ix

                oh1 = tl(rt, [P, CAP], F32, tag="oh1")
                nc.vector.tensor_tensor(
                    out=oh1[:], in0=p1[:].to_broadcast([P, CAP]),
                    in1=iotac_t[:], op=OP.is_equal)
                oh2 = tl(rt, [P, CAP], F32, tag="oh2")
                nc.vector.tensor_tensor(
                    out=oh2[:], in0=p2[:].to_broadcast([P, CAP]),
                    in1=iotac_t[:], op=OP.is_equal)
                D = tl(rt, [P, NE * CAP], F32, tag="D")
                nc.vector.tensor_tensor(
                    out=D[:].rearrange("p (e c) -> p e c", c=CAP),
                    in0=m1[:].unsqueeze(2).to_broadcast([P, NE, CAP]),
                    in1=oh1[:].unsqueeze(1).to_broadcast([P, NE, CAP]),
                    op=OP.mult)
                D2 = tl(rt, [P, NE * CAP], F32, tag="D2")
                nc.vector.tensor_tensor(
                    out=D2[:].rearrange("p (e c) -> p e c", c=CAP),
                    in0=m2[:].unsqueeze(2).to_broadcast([P, NE, CAP]),
                    in1=oh2[:].unsqueeze(1).to_broadcast([P, NE, CAP]),
                    op=OP.mult)
                nc.vector.tensor_add(D[:], D[:], D2[:])
                nm = tl(psR, [GPT, NE * CAP], F32, tag="nm")
                nc.tensor.matmul(nm[:], lhsT=nsel_t[:], rhs=D[:],
                                 start=True, stop=True)
                nm_sb = tl(rt, [GPT, NE * CAP], F32, tag="nm_sb")
                nc.vector.tensor_copy(nm_sb[:], nm[:])
                nc.sync.dma_start(nmat_d[tt], nm_sb[:])

        # slot source-row indices (per expert / slot-tile), via DRAM relayout
        islot = [[None] * nslt for _ in range(NE)]
        with tc.tile_pool(name="ip", bufs=2) as ip:
            for e in range(NE):
                for st in range(nslt):
                    f_ = tl(ip, [spt, 1], F32, tag="f")
                    src = nmat_d[:][st * tpst:(st + 1) * tpst, :,
                                    e * CAP:(e + 1) * CAP]
                    nc.gpsimd.dma_start(f_[:], src)
                    nc.vector.tensor_scalar_add(f_[:], f_[:], gbase_ts[st][:])
                    ix = tl(pB, [spt, 1], I32, tag=f"islot{e}_{st}")
                    nc.vector.tensor_copy(ix[:], f_[:])
                    islot[e][st] = ix

        # =========================================================
        # expert FFN: gather -> einT -> w1/gelu -> w2 -> eo rows -> eobuf
        # =========================================================
        eobuf = tl(dram, [NE * GCAP, E], F16, tag="eobuf")
        if c.get("debug"):
            dbg_ein = dt_("dbg_ein", [NE * GCAP, E], F32,
                          kind="ExternalOutput")[:]
            dbg_ht = dt_("dbg_ht", [HID, GCAP], F16, kind="ExternalOutput")[:]
        rrows = ybuf[:]
        with tc.tile_pool(name="einp", bufs=3) as einp, \
             tc.tile_pool(name="eintp", bufs=2) as eintp, \
             tc.tile_pool(name="wf", bufs=4) as wf, \
             tc.tile_pool(name="htp", bufs=1) as htp, \
             tc.tile_pool(name="eop", bufs=3) as eop, \
             tc.tile_pool(name="psF", bufs=1, space="PSUM") as psF, \
             tc.tile_pool(name="psT", bufs=2, space="PSUM") as psT:
            for e in range(NE):
                eins = []
                for st in range(nslt):
                    g_ = tl(einp, [spt, E], F32, tag="g")
                    nc.gpsimd.indirect_dma_start(
                        out=g_[:], out_offset=None, in_=rrows,
                        in_offset=bass.IndirectOffsetOnAxis(
                            ap=islot[e][st][:, :1], axis=0))
                    eins.append(g_)
                    if c.get("debug"):
                        nc.sync.dma_start(
                            dbg_ein[e * GCAP + st * P:e * GCAP + st * P + spt, :],
                            g_[:])
                einT = []
                for k in range(KT):
                    t_ = tl(eintp, [P, GCAP], F16, tag=f"einT{k}")
                    for st in range(nslt):
                        tp3 = tl(psT, [P, P], F32, tag="tp3")
                        nc.tensor.transpose(tp3[:, 0:spt],
                                            eins[st][:, k * P:(k + 1) * P],
                                            ident[0:spt, 0:spt])
                        nc.vector.tensor_copy(t_[:, st * P:st * P + spt],
                                              tp3[:, 0:spt])
                    einT.append(t_)
                hts = []
                for mg in range(HIDT // G1):
                    pss = [tl(psF, [P, GCAP], F32, tag=f"ps{j}")
                           for j in range(G1)]
                    for k in range(KT):
                        w1t = tl(wf, [P, G1 * P], F16, tag="w1t")
                        nc.sync.dma_start(
                            w1t[:], w1[e][k * P:(k + 1) * P,
                                          mg * G1 * P:(mg + 1) * G1 * P])
                        for j in range(G1):
                            nc.tensor.matmul(
                                pss[j][:], lhsT=w1t[:, j * P:(j + 1) * P],
                                rhs=einT[k][:], start=(k == 0),
                                stop=(k == KT - 1))
                    for j in range(G1):
                        ht_ = tl(htp, [P, GCAP], F16, tag=f"ht{mg * G1 + j}")
                        nc.scalar.activation(ht_[:], pss[j][:],
                                             getattr(ACT, c.get("act", "Gelu")))
                        hts.append(ht_)
                        if c.get("debug") and e == 0:
                            kh = mg * G1 + j
                            nc.sync.dma_start(
                                dbg_ht[kh * P:(kh + 1) * P, :], ht_[:])
                for mg in range(KT // G2):
                    pss = [tl(psF, [P, GCAP], F32, tag=f"ps{j}")
                           for j in range(G2)]
                    for kh in range(HIDT):
                        w2t = tl(wf, [P, G2 * P], F16, tag="w2t")
                        nc.sync.dma_start(
                            w2t[:], w2[e][kh * P:(kh + 1) * P,
                                          mg * G2 * P:(mg + 1) * G2 * P])
                        for j in range(G2):
                            nc.tensor.matmul(
                                pss[j][:], lhsT=w2t[:, j * P:(j + 1) * P],
                                rhs=hts[kh][:], start=(kh == 0),
                                stop=(kh == HIDT - 1))
                    for j in range(G2):
                        mE = mg * G2 + j
                        eoT_sb = tl(eop, [P, GCAP], F16, tag="eoT")
                        nc.vector.tensor_copy(eoT_sb[:], pss[j][:])
                        for st in range(nslt):
                            tp4 = tl(psT, [P, P], F16, tag="tp4")
                            nc.tensor.transpose(
                                tp4[0:spt, :], eoT_sb[:, st * P:st * P + spt],
                                ident16[0:P, 0:P])
                            eo_ = tl(eop, [spt, P], F16, tag="eo")
                            nc.vector.tensor_copy(eo_[:], tp4[0:spt, :])
                            nc.sync.dma_start(
                                eobuf[e * GCAP + st * P:e * GCAP + st * P + spt,
                                      mE * P:(mE + 1) * P], eo_[:])

        # =========================================================
        # combine + LN2 -> out
        # =========================================================
        with tc.tile_pool(name="cb", bufs=2) as cb:
            ln2g_sb = tl(pB, [P, E], F32, tag="ln2g")
            nc.gpsimd.dma_start(ln2g_sb[:], ln2g)
            ln2b_sb = tl(pB, [P, E], F32, tag="ln2b")
            nc.gpsimd.dma_start(ln2b_sb[:], ln2b)
            for tt in range(NTOKT):
                o1 = tl(cb, [P, E], F16, tag="o1")
                nc.gpsimd.indirect_dma_start(
                    out=o1[:], out_offset=None, in_=eobuf[:],
                    in_offset=bass.IndirectOffsetOnAxis(ap=idx1[tt][:, :1], axis=0))
                o2 = tl(cb, [P, E], F16, tag="o2")
                nc.gpsimd.indirect_dma_start(
                    out=o2[:], out_offset=None, in_=eobuf[:],
                    in_offset=bass.IndirectOffsetOnAxis(ap=idx2[tt][:, :1], axis=0))
                ysb = tl(cb, [P, E], F32, tag="ysb")
                nc.gpsimd.dma_start(ysb[:], ybuf[tt * P:(tt + 1) * P, :])
                s1 = tl(cb, [P, E], F32, tag="s1")
                nc.vector.tensor_scalar_mul(s1[:], o1[:], gc1[tt][:])
                s2 = tl(cb, [P, E], F32, tag="s2")
                nc.vector.tensor_scalar_mul(s2[:], o2[:], gc2[tt][:])
                z = tl(cb, [P, E], F32, tag="z")
                nc.vector.tensor_add(z[:], s1[:], s2[:])
                nc.vector.tensor_add(z[:], z[:], ysb[:])
                mu = tl(cb, [P, 1], F32, tag="mu")
                nc.vector.reduce_sum(mu[:], z[:], axis=AX.X)
                nc.vector.tensor_scalar_mul(mu[:], mu[:], 1.0 / E)
                xc = tl(cb, [P, E], F32, tag="xc")
                nc.vector.tensor_scalar(out=xc[:], in0=z[:], scalar1=mu[:],
                                        scalar2=None, op0=OP.subtract)
                scr = tl(cb, [P, E], F32, tag="scr")
                ssq = tl(cb, [P, 1], F32, tag="ssq")
                nc.scalar.activation(scr[:], xc[:], ACT.Square, accum_out=ssq[:])
                nc.vector.tensor_scalar(out=ssq[:], in0=ssq[:], scalar1=1.0 / E,
                                        scalar2=1e-5, op0=OP.mult, op1=OP.add)
                nc.scalar.sqrt(ssq[:], ssq[:])
                rstd = tl(cb, [P, 1], F32, tag="rstd")
                nc.vector.reciprocal(rstd[:], ssq[:])
                nc.vector.tensor_scalar_mul(xc[:], xc[:], rstd[:])
                yo = tl(cb, [P, E], F32, tag="yo")
                nc.vector.tensor_mul(yo[:], xc[:], ln2g_sb[:])
                nc.vector.tensor_add(yo[:], yo[:], ln2b_sb[:])
                nc.sync.dma_start(out[tt * P:(tt + 1) * P, :], yo[:])

        if c.get("debug"):
            dbg_rp = dt_("dbg_rperm", [TOK, ROW], F32, kind="ExternalOutput")[:]
            nc.sync.dma_start(dbg_rp[:, 0:E], ybuf[:])
            nc.sync.dma_start(dbg_rp[:, E:E + 8], lgbuf[:])
            dbg_eo = dt_("dbg_eobuf", [NE * GCAP, E], F16,
                         kind="ExternalOutput")[:]
            nc.sync.dma_start(dbg_eo, eobuf[:])
            dbg_ix = dt_("dbg_idx", [TOK, 2], I32, kind="ExternalOutput")[:]
            dbg_gc = dt_("dbg_gc", [TOK, 2], F32, kind="ExternalOutput")[:]
            dbg_is = dt_("dbg_islot", [NE * GCAP, 1], I32,
                         kind="ExternalOutput")[:]
            for tt in range(NTOKT):
                nc.sync.dma_start(dbg_ix[tt * P:(tt + 1) * P, 0:1], idx1[tt][:])
                nc.sync.dma_start(dbg_ix[tt * P:(tt + 1) * P, 1:2], idx2[tt][:])
                nc.sync.dma_start(dbg_gc[tt * P:(tt + 1) * P, 0:1], gc1[tt][:])
                nc.sync.dma_start(dbg_gc[tt * P:(tt + 1) * P, 1:2], gc2[tt][:])
            for e in range(NE):
                for st in range(nslt):
                    nc.sync.dma_start(
                        dbg_is[e * GCAP + st * P:e * GCAP + st * P + spt, :],
                        islot[e][st][:])

    nc.compile()
    return nc


# =========================================================
# host side
# =========================================================
_CACHE = {}


def host_prep(cfg, inputs):
    """Full (unsharded) inputs -> list of per-core input maps."""
    E = cfg["E"]
    x = np.asarray(inputs["x"], np.float32)
    t = np.asarray(inputs["time"], np.float32)
    shared = dict(
        wqkvT=np.ascontiguousarray(np.asarray(inputs["w_qkv"], np.float32).T),
        bqk=np.ascontiguousarray(
            np.asarray(inputs["b_qkv"], np.float32)[:2 * E, None]),
        bvrep=np.ascontiguousarray(
            np.tile(np.asarray(inputs["b_qkv"], np.float32)[None, 2 * E:], (P, 1))),
        woutT=np.ascontiguousarray(np.asarray(inputs["w_out"], np.float32).T),
        bout=np.ascontiguousarray(np.asarray(inputs["b_out"], np.float32)[:, None]),
        ln1g=np.ascontiguousarray(np.asarray(inputs["ln1_g"], np.float32)[:, None]),
        ln1b=np.ascontiguousarray(np.asarray(inputs["ln1_b"], np.float32)[:, None]),
        ln2grep=np.ascontiguousarray(
            np.tile(np.asarray(inputs["ln2_g"], np.float32)[None, :], (P, 1))),
        ln2brep=np.ascontiguousarray(
            np.tile(np.asarray(inputs["ln2_b"], np.float32)[None, :], (P, 1))),
        gatew=np.ascontiguousarray(np.asarray(inputs["gate_w"], np.float32)),
        w1=np.ascontiguousarray(np.asarray(inputs["w1"]).astype(np.float16)),
        w2=np.ascontiguousarray(np.asarray(inputs["w2"]).astype(np.float16)),
    )
    in_maps = []
    for cid in range(cfg["NC"]):
        m = dict(shared)
        m["xT"] = np.ascontiguousarray(x[:, cid, :].T)
        m["tcol"] = np.ascontiguousarray(t[:, cid][:, None])
        m["trep"] = np.ascontiguousarray(np.tile(t[:, cid][None, :], (P, 1)))
        in_maps.append(m)
    return in_maps


def assemble(cfg, results):
    """Per-core 'out' (TOK, E) -> full (L, B, E)."""
    L, B, E, LC = cfg["L"], cfg["B"], cfg["E"], cfg["LC"]
    full = np.empty((L, B, E), np.float32)
    for cid in range(cfg["NC"]):
        o = np.asarray(results[cid]["out"]).reshape(LC, B, E)
        full[cid * LC:(cid + 1) * LC, :, :] = o
    return full


def get_built():
    if "full" not in _CACHE:
        cfg = make_cfg(FULL)
        _CACHE["full"] = (build_bass(cfg), cfg)
    return _CACHE["full"]


def kernel(**inputs):
    nc, cfg = get_built()
    in_maps = host_prep(cfg, inputs)
    res = run_bass_kernel_spmd(nc, in_maps, core_ids=list(range(cfg["NC"])))
    return assemble(cfg, res.results)



# revision 16
# speedup vs baseline: 1.4038x; 1.4038x over previous
"""Trainium2 Bass kernel: temporal-masked MHA + top2-gated MoE layer (8 NeuronCores).

Strategy (v2):
  - data-parallel attention over B (8 batches -> 8 cores), transposed layout,
    f16 matmul inputs (psum f32), block-causal skipping (time is sorted along
    L, so the temporal mask is block-causal; the diagonal blocks still use the
    real time comparison)
  - gate logits ride a separate tiny AllToAll so top-2 routing overlaps the
    main f16 y AllToAll
  - expert FFN: w1 weight-stationary -> hT, w2 activation-stationary (hts as
    lhsT) producing eo rows directly (no output transposes); big weight DMAs
    spread across engines; double-buffered psum
"""

import math
from contextlib import ExitStack

import numpy as np

import concourse.bass as bass
import concourse.bacc as bacc
import concourse.mybir as mybir
import concourse.tile as tile
from concourse.bass_utils import run_bass_kernel_spmd
from concourse.masks import make_identity

F32 = mybir.dt.float32
F32R = mybir.dt.float32r
F16 = mybir.dt.float16
I32 = mybir.dt.int32
AX = mybir.AxisListType
OP = mybir.AluOpType
ACT = mybir.ActivationFunctionType
P = 128

FULL = dict(L=512, B=8, E=1024, H=16, HID=4096, NE=5, NC=8)


def make_cfg(d):
    c = dict(d)
    c["CAP"] = max(min(c["B"], int(c["B"] * 2.0 / c["NE"])), 4)
    c["D"] = c["E"] // c["H"]
    assert c["D"] == 64, "head dim assumed 64"
    assert c["B"] == c["NC"]
    c["LC"] = c["L"] // c["NC"]          # L-groups per core
    c["TOK"] = c["LC"] * c["B"]          # MoE tokens per core
    assert c["TOK"] % P == 0
    assert c["L"] % P == 0
    assert c["NE"] <= 8
    c["GCAP"] = c["LC"] * c["CAP"]       # slots per expert per core
    return c


def tl(pool, shape, dtype=F32, *, tag, bufs=None):
    return pool.tile(list(shape), dtype, tag=tag, name=tag, bufs=bufs)


def build_bass(c):
    nc = bacc.Bacc("TRN2", target_bir_lowering=False, debug=False,
                   num_devices=c["NC"])
    L, B, E, H, HID, NE = c["L"], c["B"], c["E"], c["H"], c["HID"], c["NE"]
    CAP, LC, TOK, GCAP = c["CAP"], c["LC"], c["TOK"], c["GCAP"]
    KT = E // P                       # k-tiles over E
    MTOK = L // P                     # token tiles (attention, per batch)
    NTOKT = TOK // P                  # token tiles (MoE)
    NHT = 2 * E // P                  # qk row tiles
    HIDT = HID // P
    GPT = P // B                      # groups per 128-token tile
    spt = min(P, GCAP)                # slots per slot-tile
    nslt = (GCAP + P - 1) // P        # slot tiles per expert
    tpst = spt // (GPT * CAP)         # token-tiles per slot-tile
    sc = 1.0 / math.sqrt(64)

    # ---- I/O ----
    dt_ = nc.dram_tensor
    xT = dt_("xT", [E, L], F32, kind="ExternalInput")[:]
    tcol = dt_("tcol", [L, 1], F32, kind="ExternalInput")[:]
    trep = dt_("trep", [P, L], F32, kind="ExternalInput")[:]
    wqkvT = dt_("wqkvT", [E, 3 * E], F16, kind="ExternalInput")[:]
    bqk = dt_("bqk", [2 * E, 1], F32, kind="ExternalInput")[:]
    bvrep = dt_("bvrep", [P, E], F32, kind="ExternalInput")[:]
    woutT = dt_("woutT", [E, E], F16, kind="ExternalInput")[:]
    bout = dt_("bout", [E, 1], F32, kind="ExternalInput")[:]
    ln1g = dt_("ln1g", [E, 1], F32, kind="ExternalInput")[:]
    ln1b = dt_("ln1b", [E, 1], F32, kind="ExternalInput")[:]
    ln2g = dt_("ln2grep", [P, E], F32, kind="ExternalInput")[:]
    ln2b = dt_("ln2brep", [P, E], F32, kind="ExternalInput")[:]
    gw = dt_("gatew", [E, NE], F32, kind="ExternalInput")[:]
    w1 = dt_("w1", [NE, E, HID], F16, kind="ExternalInput")[:]
    w2 = dt_("w2", [NE, HID, E], F16, kind="ExternalInput")[:]
    out = dt_("out", [TOK, E], F32, kind="ExternalOutput")[:]

    # ---- host-side constant tables (baked into the NEFF) ----
    tri = np.zeros((P, P), np.float32)       # strict-lower within B-groups
    ob = np.zeros((P, P), np.float32)        # all-ones within B-groups
    for i in range(P):
        for j in range(P):
            if i // B == j // B:
                ob[i, j] = 1.0
                if i < j:
                    tri[i, j] = 1.0
    nsel = np.zeros((P, GPT), np.float32)
    for i in range(P):
        nsel[i, i // B] = float(i % B)
    iotac = np.tile(np.arange(CAP, dtype=np.float32), (P, 1))
    iotae = np.tile(np.arange(NE, dtype=np.float32), (P, 1))
    gbase = np.zeros((nslt, spt, 1), np.float32)
    for st in range(nslt):
        for p in range(spt):
            gbase[st, p, 0] = float(B * ((st * P + p) // CAP))
    gb2 = np.zeros((NTOKT, P, 1), np.float32)
    for t in range(NTOKT):
        for p in range(P):
            gb2[t, p, 0] = float(CAP * ((t * P + p) // B))

    # round-robin engines for weight-stream DMA triggering (vector cannot
    # trigger DMAs; gpsimd's collectives/gathers all precede these in
    # program order so its queue has slack during the FFN)
    dma_engines = [nc.sync, nc.scalar, nc.gpsimd]
    _ecnt = [0]

    def wdma(dst, src):
        e = dma_engines[_ecnt[0] % len(dma_engines)]
        _ecnt[0] += 1
        e.dma_start(dst, src)

    with tile.TileContext(nc) as tc, ExitStack() as ctx:
        cst = ctx.enter_context(tc.tile_pool(name="cst", bufs=1))
        dram = ctx.enter_context(tc.tile_pool(name="dram", bufs=1, space="DRAM"))
        pB = ctx.enter_context(tc.tile_pool(name="pB", bufs=1))

        def const_tile(arr, tag):
            ap = nc.inline_tensor(np.ascontiguousarray(arr), name=tag)[:]
            t = tl(cst, list(arr.shape), F32, tag=tag)
            nc.gpsimd.dma_start(t[:], ap)
            return t

        ident = tl(cst, [P, P], F32, tag="ident")
        make_identity(nc, ident[:])
        ident16 = tl(cst, [P, P], F16, tag="ident16")
        make_identity(nc, ident16[:])
        ones_t = tl(cst, [P, 1], F32, tag="ones")
        nc.vector.memset(ones_t[:], 1.0)
        onesr_t = tl(cst, [1, P], F32, tag="onesr")
        nc.vector.memset(onesr_t[:], 1.0)
        onesr16 = tl(cst, [1, P], F16, tag="onesr16")
        nc.vector.memset(onesr16[:], 1.0)
        ones16 = tl(cst, [P, 1], F16, tag="ones16")
        nc.vector.memset(ones16[:], 1.0)
        tri_t = const_tile(tri, "tri")
        ob_t = const_tile(ob, "ob")
        nsel_t = const_tile(nsel, "nsel")
        iotac_t = const_tile(iotac, "iotac")
        iotae_t = const_tile(iotae, "iotae")
        gbase_ap = nc.inline_tensor(gbase, name="gbase")[:]
        gbase_ts = []
        for st in range(nslt):
            t = tl(cst, [spt, 1], F32, tag=f"gbase{st}")
            nc.gpsimd.dma_start(t[:], gbase_ap[st])
            gbase_ts.append(t)
        gb2_ap = nc.inline_tensor(gb2, name="gb2")[:]
        gb2_ts = []
        for tt in range(NTOKT):
            t = tl(cst, [P, 1], F32, tag=f"gb2{tt}")
            nc.gpsimd.dma_start(t[:], gb2_ap[tt])
            gb2_ts.append(t)

        send16 = tl(dram, [L, E], F16, tag="send16")
        sendlg = tl(dram, [L, 8], F32, tag="sendlg")

        # =========================================================
        # PHASE A: attention for this core's batch (transposed layout, f16)
        # =========================================================
        yT = []
        y16 = []
        with tc.tile_pool(name="pA", bufs=1) as pA:
            xt = []
            xt16 = []
            for k in range(KT):
                t = tl(pA, [P, L], F32, tag=f"xt{k}")
                nc.gpsimd.dma_start(t[:], xT[k * P:(k + 1) * P, :])
                xt.append(t)
                t16 = tl(pA, [P, L], F16, tag=f"xt16_{k}")
                nc.vector.tensor_copy(t16[:], t[:])
                xt16.append(t16)
            bqk_t = tl(pA, [P, NHT], F32, tag="bqk")
            nc.gpsimd.dma_start(bqk_t[:], bqk.rearrange("(m p) o -> p (m o)", p=P))
            bv_t = []
            for nn in range(2):
                t = tl(pA, [P, 512], F32, tag=f"bv{nn}")
                nc.gpsimd.dma_start(t[:], bvrep[:, nn * 512:(nn + 1) * 512])
                bv_t.append(t)
            tcol_t = tl(pA, [P, MTOK], F32, tag="tcol")
            nc.gpsimd.dma_start(tcol_t[:], tcol.rearrange("(m p) o -> p (m o)", p=P))
            trep_t = tl(pA, [P, L], F32, tag="trep")
            nc.gpsimd.dma_start(trep_t[:], trep)
            gw_t = tl(pA, [P, KT * NE], F32, tag="gw")
            nc.sync.dma_start(gw_t[:].rearrange("p (k e) -> p k e", e=NE),
                              gw.rearrange("(k p) e -> p k e", p=P))
            bout_t = tl(pA, [P, KT], F32, tag="bout")
            nc.gpsimd.dma_start(bout_t[:], bout.rearrange("(m p) o -> p (m o)", p=P))
            ln1g_t = tl(pA, [P, KT], F32, tag="ln1g")
            nc.gpsimd.dma_start(ln1g_t[:], ln1g.rearrange("(m p) o -> p (m o)", p=P))
            ln1b_t = tl(pA, [P, KT], F32, tag="ln1b")
            nc.gpsimd.dma_start(ln1b_t[:], ln1b.rearrange("(m p) o -> p (m o)", p=P))

            # ---- qkT = wqkv[:2E] @ x^T (f16); V token-major w/ ones col
            qk = [tl(pA, [P, L], F16, tag=f"qk{m}") for m in range(NHT)]
            VW = H * 65
            vt = [tl(pA, [P, VW], F16, tag=f"vt{m}") for m in range(MTOK)]
            with tc.tile_pool(name="wp", bufs=2) as wp, \
                 tc.tile_pool(name="psQ", bufs=2, space="PSUM") as psQ:
                for mg in range(NHT // 4):
                    wq = []
                    for k in range(KT):
                        t = tl(wp, [P, 512], F16, tag=f"wq{k}")
                        nc.sync.dma_start(
                            t[:], wqkvT[k * P:(k + 1) * P,
                                        mg * 512:(mg + 1) * 512])
                        wq.append(t)
                    for j in range(4):
                        m = mg * 4 + j
                        ps = tl(psQ, [P, L], F32, tag="ps")
                        for k in range(KT):
                            nc.tensor.matmul(
                                ps[:], lhsT=wq[k][:, j * P:(j + 1) * P],
                                rhs=xt16[k][:], start=(k == 0),
                                stop=(k == KT - 1))
                        nc.vector.tensor_scalar_add(qk[m][:], ps[:],
                                                    bqk_t[:, m:m + 1])
                # V: x-stationary, weight cols streamed
                wv = {}
                for nn in range(2):
                    for k in range(KT):
                        t = tl(wp, [P, 512], F16, tag=f"wv{nn}_{k}", bufs=1)
                        nc.sync.dma_start(
                            t[:], wqkvT[k * P:(k + 1) * P,
                                        2 * E + nn * 512:2 * E + (nn + 1) * 512])
                        wv[(nn, k)] = t
                for mt in range(MTOK):
                    for nn in range(2):
                        ps = tl(psQ, [P, 512], F32, tag="ps")
                        for k in range(KT):
                            nc.tensor.matmul(
                                ps[:], lhsT=xt16[k][:, mt * P:(mt + 1) * P],
                                rhs=wv[(nn, k)][:], start=(k == 0),
                                stop=(k == KT - 1))
                        dst = vt[mt][:].rearrange("p (h e) -> p h e", e=65)[
                            :, nn * 8:(nn + 1) * 8, 0:64]
                        nc.vector.tensor_add(
                            dst, ps[:].rearrange("p (h e) -> p h e", e=64),
                            bv_t[nn][:].rearrange("p (h e) -> p h e", e=64))
                    nc.vector.memset(
                        vt[mt][:].rearrange("p (h e) -> p h e", e=65)[:, :, 64:65],
                        1.0)

            # ---- diagonal temporal masks (0 / -8e9; exp scale 1/8 -> -1e9)
            maskd = [tl(pA, [P, P], F32, tag=f"md{kt}") for kt in range(MTOK)]
            for kt in range(MTOK):
                nc.vector.tensor_tensor(
                    out=maskd[kt][:],
                    in0=tcol_t[:, kt:kt + 1].to_broadcast([P, P]),
                    in1=trep_t[:, kt * P:(kt + 1) * P], op=OP.is_gt)
                nc.vector.tensor_scalar_mul(maskd[kt][:], maskd[kt][:],
                                            -1e9 / sc)

            # ---- heads (block-causal: key tile kt only attends q >= kt*P)
            attnT = [tl(pA, [P, L], F16, tag=f"at{k}") for k in range(KT)]
            with tc.tile_pool(name="pp", bufs=2) as pp, \
                 tc.tile_pool(name="smp", bufs=3) as smp, \
                 tc.tile_pool(name="psS", bufs=1, space="PSUM") as psS, \
                 tc.tile_pool(name="psAV", bufs=2, space="PSUM") as psAV:
                for h in range(H):
                    mq, rq = (h * 64) // P, (h * 64) % P
                    mk, rk = (E + h * 64) // P, (E + h * 64) % P
                    pts = []
                    for kt in range(MTOK):
                        N = L - kt * P
                        sps = tl(psS, [P, N], F32, tag=f"sps{kt}")
                        nc.tensor.matmul(
                            sps[:],
                            lhsT=qk[mk][rk:rk + 64, kt * P:(kt + 1) * P],
                            rhs=qk[mq][rq:rq + 64, kt * P:L],
                            start=True, stop=True)
                        nc.vector.tensor_add(sps[:, 0:P], sps[:, 0:P],
                                             maskd[kt][:])
                        pt_ = tl(pp, [P, N], F16, tag=f"pt{kt}")
                        nc.scalar.activation(pt_[:], sps[:], ACT.Exp, scale=sc)
                        pts.append(pt_)
                    av = tl(psAV, [65, L], F32, tag="av")
                    for qt in range(MTOK):
                        for kt in range(qt + 1):
                            nc.tensor.matmul(
                                av[:, qt * P:(qt + 1) * P],
                                lhsT=vt[kt][:, h * 65:h * 65 + 65],
                                rhs=pts[kt][:, (qt - kt) * P:(qt - kt + 1) * P],
                                start=(kt == 0), stop=(kt == qt))
                    rec = tl(smp, [1, L], F32, tag="rec")
                    nc.vector.reciprocal(rec[:], av[64:65, :])
                    rec16 = tl(smp, [1, L], F16, tag="rec16")
                    nc.vector.tensor_copy(rec16[:], rec[:])
                    rep_ps = tl(psAV, [64, L], F32, tag="repps")
                    nc.tensor.matmul(rep_ps[:], lhsT=onesr16[:, 0:64],
                                     rhs=rec16[:], start=True, stop=True)
                    rep = tl(smp, [64, L], F16, tag="rep")
                    nc.vector.tensor_copy(rep[:], rep_ps[:])
                    nc.vector.tensor_mul(attnT[mq][rq:rq + 64, :], av[0:64, :],
                                         rep[:])

            # ---- out-proj + residual (into xt -> zT)
            with tc.tile_pool(name="wp2", bufs=2) as wp2, \
                 tc.tile_pool(name="psO", bufs=2, space="PSUM") as psO:
                for mg in range(KT // 4):
                    wo = []
                    for k in range(KT):
                        t = tl(wp2, [P, 512], F16, tag=f"wo{k}")
                        nc.sync.dma_start(
                            t[:], woutT[k * P:(k + 1) * P,
                                        mg * 512:(mg + 1) * 512])
                        wo.append(t)
                    for j in range(4):
                        m = mg * 4 + j
                        ps = tl(psO, [P, L], F32, tag="ps")
                        for k in range(KT):
                            nc.tensor.matmul(ps[:],
                                             lhsT=wo[k][:, j * P:(j + 1) * P],
                                             rhs=attnT[k][:],
                                             start=(k == 0), stop=(k == KT - 1))
                        nc.vector.tensor_scalar_add(ps[:], ps[:],
                                                    bout_t[:, m:m + 1])
                        nc.vector.tensor_add(xt[m][:], ps[:], xt[m][:])  # zT

            # ---- LN1 -> yT (f32) + y16 (f16)
            for k in range(KT):
                yT.append(tl(pA, [P, L], F32, tag=f"yT{k}"))
                y16.append(tl(pA, [P, L], F16, tag=f"y16_{k}"))
            with tc.tile_pool(name="lnp", bufs=3) as lnp, \
                 tc.tile_pool(name="psL", bufs=1, space="PSUM") as psL:
                mu_ps = tl(psL, [1, L], F32, tag="mu")
                sq_ps = tl(psL, [1, L], F32, tag="sq")
                for k in range(KT):
                    z16 = tl(lnp, [P, L], F16, tag="z16")
                    nc.vector.tensor_copy(z16[:], xt[k][:])
                    nc.tensor.matmul(mu_ps[:], lhsT=ones16[:], rhs=z16[:],
                                     start=(k == 0), stop=(k == KT - 1))
                    sqt = tl(lnp, [P, L], F16, tag="sqt")
                    nc.scalar.activation(sqt[:], xt[k][:], ACT.Square)
                    nc.tensor.matmul(sq_ps[:], lhsT=ones16[:], rhs=sqt[:],
                                     start=(k == 0), stop=(k == KT - 1))
                mu_r = tl(lnp, [1, L], F32, tag="mu_r")
                nc.vector.tensor_scalar_mul(mu_r[:], mu_ps[:], 1.0 / E)
                var_r = tl(lnp, [1, L], F32, tag="var_r")
                nc.vector.tensor_scalar_mul(var_r[:], sq_ps[:], 1.0 / E)
                mu2 = tl(lnp, [1, L], F32, tag="mu2")
                nc.vector.tensor_mul(mu2[:], mu_r[:], mu_r[:])
                nc.vector.tensor_sub(var_r[:], var_r[:], mu2[:])
                nc.vector.tensor_scalar_add(var_r[:], var_r[:], 1e-5)
                nc.scalar.sqrt(var_r[:], var_r[:])
                rstd_r = tl(lnp, [1, L], F32, tag="rstd_r")
                nc.vector.reciprocal(rstd_r[:], var_r[:])
                mur16 = tl(lnp, [1, L], F16, tag="mur16")
                nc.vector.tensor_copy(mur16[:], mu_r[:])
                rstdr16 = tl(lnp, [1, L], F16, tag="rstdr16")
                nc.vector.tensor_copy(rstdr16[:], rstd_r[:])
                murep_ps = tl(psL, [P, L], F32, tag="murep")
                nc.tensor.matmul(murep_ps[:], lhsT=onesr16[:],
                                 rhs=mur16[:], start=True, stop=True)
                mu_rep = tl(lnp, [P, L], F32, tag="mu_rep")
                nc.vector.tensor_copy(mu_rep[:], murep_ps[:])
                rsrep_ps = tl(psL, [P, L], F32, tag="rsrep")
                nc.tensor.matmul(rsrep_ps[:], lhsT=onesr16[:],
                                 rhs=rstdr16[:], start=True, stop=True)
                rstd_rep = tl(lnp, [P, L], F32, tag="rstd_rep")
                nc.vector.tensor_copy(rstd_rep[:], rsrep_ps[:])
                for k in range(KT):
                    t1 = tl(lnp, [P, L], F32, tag="t1")
                    nc.vector.tensor_sub(t1[:], xt[k][:], mu_rep[:])
                    nc.vector.tensor_mul(t1[:], t1[:], rstd_rep[:])
                    nc.vector.tensor_scalar(
                        out=yT[k][:], in0=t1[:], scalar1=ln1g_t[:, k:k + 1],
                        scalar2=ln1b_t[:, k:k + 1], op0=OP.mult, op1=OP.add)
                    nc.vector.tensor_copy(y16[k][:], yT[k][:])

            # ---- gate logits -> sendlg (f32, small)
            with tc.tile_pool(name="gp", bufs=3) as gp, \
                 tc.tile_pool(name="psG", bufs=2, space="PSUM") as psG:
                gps = tl(psG, [NE, L], F32, tag="gps")
                for k in range(KT):
                    nc.tensor.matmul(
                        gps[:], lhsT=gw_t[:, k * NE:(k + 1) * NE],
                        rhs=yT[k][:],
                        start=(k == 0), stop=(k == KT - 1))
                lg_sb = tl(gp, [NE, L], F32, tag="lg_sb")
                nc.vector.tensor_copy(lg_sb[:], gps[:])
                for ct in range(MTOK):
                    tp2 = tl(psG, [P, NE], F32, tag="tp2")
                    nc.tensor.transpose(tp2[:, 0:NE],
                                        lg_sb[:, ct * P:(ct + 1) * P],
                                        ident[0:NE, 0:NE])
                    lgr = tl(gp, [P, 8], F32, tag="lgr")
                    nc.vector.memset(lgr[:, NE:8], 0.0)
                    nc.vector.tensor_copy(lgr[:, 0:NE], tp2[:, 0:NE])
                    nc.sync.dma_start(sendlg[ct * P:(ct + 1) * P, :], lgr[:])

                # tiny logits AllToAll first: lets routing overlap the y A2A
                recvlg = tl(dram, [L, 8], F32, tag="recvlg")
                nc.gpsimd.collective_compute(
                    "AllToAll", OP.bypass,
                    replica_groups=[list(range(c["NC"]))],
                    ins=[sendlg[:].opt()], outs=[recvlg[:].opt()])

                # ---- y -> token-major f16 send buffer
                for ct in range(MTOK):
                    yrow = tl(gp, [P, E], F16, tag="yrow")
                    for k in range(KT):
                        tpY = tl(psG, [P, P], F16, tag="tpY")
                        nc.tensor.transpose(tpY[:], y16[k][:, ct * P:(ct + 1) * P],
                                            ident16[:])
                        nc.vector.tensor_copy(yrow[:, k * P:(k + 1) * P], tpY[:])
                    nc.sync.dma_start(send16[ct * P:(ct + 1) * P, :], yrow[:])

        # FFN weight pools open once attention SBUF is released; expert-0 w1
        # prefetch (tag per k, bufs=2: 8 tags x 2 x 4KB/part = 64KB/part)
        # streams during the A2A + routing window
        wf = ctx.enter_context(tc.tile_pool(name="wf", bufs=2))
        w2p = ctx.enter_context(tc.tile_pool(name="w2p", bufs=3))
        w1pre = {}
        for half in range(2):
            for k in range(KT):
                t = tl(wf, [P, 2048], F16, tag=f"w1_{k}")
                nc.sync.dma_start(
                    t[:], w1[0][k * P:(k + 1) * P, half * 2048:(half + 1) * 2048])
                w1pre[(half, k)] = t

        # =========================================================
        # AllToAll + permute to group-major token order
        # =========================================================
        recv16 = tl(dram, [L, E], F16, tag="recv16")
        nc.gpsimd.collective_compute(
            "AllToAll", OP.bypass,
            replica_groups=[list(range(c["NC"]))],
            ins=[send16[:].opt()], outs=[recv16[:].opt()])
        lgbuf = tl(dram, [TOK, 8], F32, tag="lgbuf")
        nc.sync.dma_start(
            lgbuf[:].rearrange("(l i) r -> l i r", i=c["NC"]),
            recvlg[:].rearrange("(i l) r -> l i r", i=c["NC"]))

        # =========================================================
        # PHASE B: top-2 routing with capacity (overlaps the y A2A)
        # =========================================================
        nmat_d = tl(dram, [NTOKT, GPT, NE * CAP], F32, tag="nmat_d")
        gc1 = [None] * NTOKT
        gc2 = [None] * NTOKT
        idx1 = [None] * NTOKT
        idx2 = [None] * NTOKT
        with tc.tile_pool(name="rt", bufs=2) as rt, \
             tc.tile_pool(name="psR", bufs=2, space="PSUM") as psR:
            for tt in range(NTOKT):
                lg = tl(rt, [P, NE], F32, tag="lg")
                nc.scalar.dma_start(lg[:], lgbuf[tt * P:(tt + 1) * P, 0:NE])
                mx = tl(rt, [P, 1], F32, tag="mx")
                nc.vector.reduce_max(mx[:], lg[:], axis=AX.X)
                nc.vector.tensor_scalar_mul(mx[:], mx[:], -1.0)
                ex = tl(rt, [P, NE], F32, tag="ex")
                sm = tl(rt, [P, 1], F32, tag="sm")
                nc.scalar.activation(ex[:], lg[:], ACT.Exp, bias=mx[:],
                                     accum_out=sm[:])
                rcp = tl(rt, [P, 1], F32, tag="rcp")
                nc.vector.reciprocal(rcp[:], sm[:])
                raw = tl(rt, [P, NE], F32, tag="raw")
                nc.vector.tensor_scalar_mul(raw[:], ex[:], rcp[:])

                def top1(rawt, tag):
                    g = tl(rt, [P, 1], F32, tag=f"g{tag}")
                    nc.vector.reduce_max(g[:], rawt[:], axis=AX.X)
                    eq = tl(rt, [P, NE], F32, tag=f"eq{tag}")
                    nc.vector.tensor_tensor(
                        out=eq[:], in0=rawt[:],
                        in1=g[:].to_broadcast([P, NE]), op=OP.is_ge)
                    cs = tl(rt, [P, NE], F32, tag=f"cs{tag}")
                    nc.vector.memset(cs[:, 0:1], 0.0)
                    for j in range(1, NE):
                        nc.vector.tensor_add(cs[:, j:j + 1], cs[:, j - 1:j],
                                             eq[:, j - 1:j])
                    fst = tl(rt, [P, NE], F32, tag=f"fst{tag}")
                    nc.vector.tensor_scalar(out=fst[:], in0=cs[:], scalar1=0.5,
                                            scalar2=None, op0=OP.is_lt)
                    m_ = tl(rt, [P, NE], F32, tag=f"m{tag}")
                    nc.vector.tensor_mul(m_[:], eq[:], fst[:])
                    return g, m_

                g1, m1r = top1(raw, "1")
                raw2 = tl(rt, [P, NE], F32, tag="raw2")
                nc.vector.tensor_mul(raw2[:], raw[:], m1r[:])
                nc.vector.tensor_sub(raw2[:], raw[:], raw2[:])
                g2, m2r = top1(raw2, "2")
                den = tl(rt, [P, 1], F32, tag="den")
                nc.vector.tensor_add(den[:], g1[:], g2[:])
                nc.vector.tensor_scalar_add(den[:], den[:], 1e-9)
                rd = tl(rt, [P, 1], F32, tag="rd")
                nc.vector.reciprocal(rd[:], den[:])
                g1n = tl(rt, [P, 1], F32, tag="g1n")
                nc.vector.tensor_mul(g1n[:], g1[:], rd[:])
                g2n = tl(rt, [P, 1], F32, tag="g2n")
                nc.vector.tensor_mul(g2n[:], g2[:], rd[:])

                pos1 = tl(psR, [P, NE], F32, tag="pos1")
                nc.tensor.matmul(pos1[:], lhsT=tri_t[:], rhs=m1r[:],
                                 start=True, stop=True)
                keep1 = tl(rt, [P, NE], F32, tag="keep1")
                nc.vector.tensor_scalar(out=keep1[:], in0=pos1[:],
                                        scalar1=CAP - 0.5, scalar2=None,
                                        op0=OP.is_lt)
                m1 = tl(rt, [P, NE], F32, tag="m1")
                nc.vector.tensor_mul(m1[:], m1r[:], keep1[:])
                pos2 = tl(psR, [P, NE], F32, tag="pos2")
                nc.tensor.matmul(pos2[:], lhsT=tri_t[:], rhs=m2r[:],
                                 start=True, stop=False)
                nc.tensor.matmul(pos2[:], lhsT=ob_t[:], rhs=m1[:],
                                 start=False, stop=True)
                keep2 = tl(rt, [P, NE], F32, tag="keep2")
                nc.vector.tensor_scalar(out=keep2[:], in0=pos2[:],
                                        scalar1=CAP - 0.5, scalar2=None,
                                        op0=OP.is_lt)
                m2 = tl(rt, [P, NE], F32, tag="m2")
                nc.vector.tensor_mul(m2[:], m2r[:], keep2[:])

                def dotX(a_ap, b_ap, tag):
                    t5 = tl(rt, [P, NE], F32, tag=f"t5{tag}")
                    nc.vector.tensor_mul(t5[:], a_ap, b_ap)
                    o = tl(rt, [P, 1], F32, tag=f"o{tag}")
                    nc.vector.reduce_sum(o[:], t5[:], axis=AX.X)
                    return o

                m1f = tl(rt, [P, 1], F32, tag="m1f")
                nc.vector.reduce_sum(m1f[:], m1[:], axis=AX.X)
                m2f = tl(rt, [P, 1], F32, tag="m2f")
                nc.vector.reduce_sum(m2f[:], m2[:], axis=AX.X)
                gc1[tt] = tl(pB, [P, 1], F32, tag=f"gc1_{tt}")
                nc.vector.tensor_mul(gc1[tt][:], g1n[:], m1f[:])
                gc2[tt] = tl(pB, [P, 1], F32, tag=f"gc2_{tt}")
                nc.vector.tensor_mul(gc2[tt][:], g2n[:], m2f[:])
                p1 = dotX(pos1[:], m1[:], "p1")
                p2 = dotX(pos2[:], m2[:], "p2")
                e1 = dotX(iotae_t[:], m1[:], "e1")
                e2 = dotX(iotae_t[:], m2[:], "e2")

                for (en, pn, sl) in ((e1, p1, 1), (e2, p2, 2)):
                    f_ = tl(rt, [P, 1], F32, tag=f"if{sl}")
                    nc.vector.tensor_scalar(
                        out=f_[:], in0=en[:], scalar1=float(GCAP),
                        scalar2=gb2_ts[tt][:], op0=OP.mult, op1=OP.add)
                    nc.vector.tensor_add(f_[:], f_[:], pn[:])
                    ix = tl(pB, [P, 1], I32, tag=f"idx{sl}_{tt}")
                    nc.vector.tensor_copy(ix[:], f_[:])
                    (idx1 if sl == 1 else idx2)[tt] = ix

                oh1 = tl(rt, [P, CAP], F32, tag="oh1")
                nc.vector.tensor_tensor(
                    out=oh1[:], in0=p1[:].to_broadcast([P, CAP]),
                    in1=iotac_t[:], op=OP.is_equal)
                oh2 = tl(rt, [P, CAP], F32, tag="oh2")
                nc.vector.tensor_tensor(
                    out=oh2[:], in0=p2[:].to_broadcast([P, CAP]),
                    in1=iotac_t[:], op=OP.is_equal)
                D = tl(rt, [P, NE * CAP], F32, tag="D")
                nc.vector.tensor_tensor(
                    out=D[:].rearrange("p (e c) -> p e c", c=CAP),
                    in0=m1[:].unsqueeze(2).to_broadcast([P, NE, CAP]),
                    in1=oh1[:].unsqueeze(1).to_broadcast([P, NE, CAP]),
                    op=OP.mult)
                D2 = tl(rt, [P, NE * CAP], F32, tag="D2")
                nc.vector.tensor_tensor(
                    out=D2[:].rearrange("p (e c) -> p e c", c=CAP),
                    in0=m2[:].unsqueeze(2).to_broadcast([P, NE, CAP]),
                    in1=oh2[:].unsqueeze(1).to_broadcast([P, NE, CAP]),
                    op=OP.mult)
                nc.vector.tensor_add(D[:], D[:], D2[:])
                nm = tl(psR, [GPT, NE * CAP], F32, tag="nm")
                nc.tensor.matmul(nm[:], lhsT=nsel_t[:], rhs=D[:],
                                 start=True, stop=True)
                nm_sb = tl(rt, [GPT, NE * CAP], F32, tag="nm_sb")
                nc.vector.tensor_copy(nm_sb[:], nm[:])
                nc.sync.dma_start(nmat_d[tt], nm_sb[:])

        # slot source-row indices (per expert / slot-tile), via DRAM relayout
        islot = [[None] * nslt for _ in range(NE)]
        with tc.tile_pool(name="ip", bufs=2) as ip:
            for e in range(NE):
                for st in range(nslt):
                    f_ = tl(ip, [spt, 1], F32, tag="f")
                    src = nmat_d[:][st * tpst:(st + 1) * tpst, :,
                                    e * CAP:(e + 1) * CAP]
                    nc.sync.dma_start(f_[:], src)
                    nc.vector.tensor_scalar_add(f_[:], f_[:], gbase_ts[st][:])
                    ix = tl(pB, [spt, 1], I32, tag=f"islot{e}_{st}")
                    nc.vector.tensor_copy(ix[:], f_[:])
                    islot[e][st] = ix

        # y A2A result -> group-major token order (issued here so the sync
        # queue is not blocked behind the big A2A during routing)
        ybuf16 = tl(dram, [TOK, E], F16, tag="ybuf16")
        nc.sync.dma_start(
            ybuf16[:].rearrange("(l i) r -> l i r", i=c["NC"]),
            recv16[:].rearrange("(i l) r -> l i r", i=c["NC"]))

        # =========================================================
        # expert FFN: gather -> einT -> w1/gelu -> w2 (act-stationary)
        # =========================================================
        eobuf = tl(dram, [NE * GCAP, E], F16, tag="eobuf")
        G1 = 2
        with tc.tile_pool(name="einp", bufs=1) as einp, \
             tc.tile_pool(name="eintp", bufs=1) as eintp, \
             tc.tile_pool(name="htp", bufs=2) as htp, \
             tc.tile_pool(name="eop", bufs=2) as eop, \
             tc.tile_pool(name="psF", bufs=2, space="PSUM") as psF, \
             tc.tile_pool(name="psW2", bufs=1, space="PSUM") as psW2, \
             tc.tile_pool(name="psT", bufs=2, space="PSUM") as psT:
            # prefetch ALL expert gathers up front (gpsimd queue)
            eins = {}
            for e in range(NE):
                for st in range(nslt):
                    g_ = tl(einp, [spt, E], F16, tag=f"g{e}_{st}")
                    nc.gpsimd.indirect_dma_start(
                        out=g_[:], out_offset=None, in_=ybuf16[:],
                        in_offset=bass.IndirectOffsetOnAxis(
                            ap=islot[e][st][:, :1], axis=0))
                    eins[(e, st)] = g_

            for e in range(NE):
                # ---- einT for this expert (just-in-time on tensor queue)
                einT = {}
                for k in range(KT):
                    t_ = tl(eintp, [P, GCAP], F16, tag=f"einT{e}_{k}")
                    for st in range(nslt):
                        tp3 = tl(psT, [P, P], F16, tag="tp3")
                        nc.tensor.transpose(tp3[:, 0:spt],
                                            eins[(e, st)][:, k * P:(k + 1) * P],
                                            ident16[0:spt, 0:spt])
                        nc.vector.tensor_copy(t_[:, st * P:st * P + spt],
                                              tp3[:, 0:spt])
                    einT[(e, k)] = t_
                # ---- w1: weight-stationary -> hT tiles [hid, slots]
                hts = []
                for half in range(2):
                    w1t = []
                    for k in range(KT):
                        if e == 0:
                            w1t.append(w1pre[(half, k)])
                        else:
                            t = tl(wf, [P, 2048], F16, tag=f"w1_{k}")
                            wdma(t[:], w1[e][k * P:(k + 1) * P,
                                             half * 2048:(half + 1) * 2048])
                            w1t.append(t)
                    for mg in range(16 // G1):
                        pss = [tl(psF, [P, GCAP], F32, tag=f"ps{j}")
                               for j in range(G1)]
                        for k in range(KT):
                            for j in range(G1):
                                col = (mg * G1 + j) * P
                                nc.tensor.matmul(
                                    pss[j][:],
                                    lhsT=w1t[k][:, col:col + P],
                                    rhs=einT[(e, k)][:], start=(k == 0),
                                    stop=(k == KT - 1))
                        for j in range(G1):
                            kh = half * 16 + mg * G1 + j
                            ht_ = tl(htp, [P, GCAP], F16, tag=f"ht{kh}")
                            nc.scalar.activation(ht_[:], pss[j][:], ACT.Gelu)
                            hts.append(ht_)
                # ---- w2: activation-stationary -> eo rows direct
                for ch in range(2):
                    pw = [tl(psW2, [P, 512], F32, tag=f"pw{sb}")
                          for sb in range(nslt)]
                    for kh in range(HIDT):
                        w2r = tl(w2p, [P, 512], F16, tag="w2r")
                        wdma(w2r[:], w2[e][kh * P:(kh + 1) * P,
                                           ch * 512:(ch + 1) * 512])
                        for sb in range(nslt):
                            nc.tensor.matmul(
                                pw[sb][:],
                                lhsT=hts[kh][:, sb * P:sb * P + spt],
                                rhs=w2r[:], start=(kh == 0),
                                stop=(kh == HIDT - 1))
                    for sb in range(nslt):
                        eo16 = tl(eop, [P, 512], F16, tag="eo16")
                        nc.vector.tensor_copy(eo16[0:spt, :], pw[sb][0:spt, :])
                        nc.sync.dma_start(
                            eobuf[e * GCAP + sb * P:e * GCAP + sb * P + spt,
                                  ch * 512:(ch + 1) * 512], eo16[0:spt, :])

        # =========================================================
        # combine + LN2 -> out
        # =========================================================
        with tc.tile_pool(name="cb", bufs=2) as cb:
            ln2g_sb = tl(pB, [P, E], F32, tag="ln2g")
            nc.gpsimd.dma_start(ln2g_sb[:], ln2g)
            ln2b_sb = tl(pB, [P, E], F32, tag="ln2b")
            nc.gpsimd.dma_start(ln2b_sb[:], ln2b)
            for tt in range(NTOKT):
                o1 = tl(cb, [P, E], F16, tag="o1")
                nc.gpsimd.indirect_dma_start(
                    out=o1[:], out_offset=None, in_=eobuf[:],
                    in_offset=bass.IndirectOffsetOnAxis(ap=idx1[tt][:, :1], axis=0))
                o2 = tl(cb, [P, E], F16, tag="o2")
                nc.gpsimd.indirect_dma_start(
                    out=o2[:], out_offset=None, in_=eobuf[:],
                    in_offset=bass.IndirectOffsetOnAxis(ap=idx2[tt][:, :1], axis=0))
                ysb = tl(cb, [P, E], F16, tag="ysb")
                nc.sync.dma_start(ysb[:], ybuf16[tt * P:(tt + 1) * P, :])
                s1 = tl(cb, [P, E], F32, tag="s1")
                nc.gpsimd.tensor_scalar_mul(s1[:], o1[:], gc1[tt][:])
                s2 = tl(cb, [P, E], F32, tag="s2")
                nc.vector.tensor_scalar_mul(s2[:], o2[:], gc2[tt][:])
                z = tl(cb, [P, E], F32, tag="z")
                nc.gpsimd.tensor_add(z[:], s1[:], s2[:])
                nc.vector.tensor_add(z[:], z[:], ysb[:])
                mu = tl(cb, [P, 1], F32, tag="mu")
                nc.vector.reduce_sum(mu[:], z[:], axis=AX.X)
                nc.vector.tensor_scalar_mul(mu[:], mu[:], 1.0 / E)
                xc = tl(cb, [P, E], F32, tag="xc")
                nc.vector.tensor_scalar(out=xc[:], in0=z[:], scalar1=mu[:],
                                        scalar2=None, op0=OP.subtract)
                scr = tl(cb, [P, E], F32, tag="scr")
                ssq = tl(cb, [P, 1], F32, tag="ssq")
                nc.scalar.activation(scr[:], xc[:], ACT.Square, accum_out=ssq[:])
                nc.vector.tensor_scalar(out=ssq[:], in0=ssq[:], scalar1=1.0 / E,
                                        scalar2=1e-5, op0=OP.mult, op1=OP.add)
                nc.scalar.sqrt(ssq[:], ssq[:])
                rstd = tl(cb, [P, 1], F32, tag="rstd")
                nc.vector.reciprocal(rstd[:], ssq[:])
                nc.vector.tensor_scalar_mul(xc[:], xc[:], rstd[:])
                yo = tl(cb, [P, E], F32, tag="yo")
                nc.gpsimd.tensor_mul(yo[:], xc[:], ln2g_sb[:])
                nc.vector.tensor_add(yo[:], yo[:], ln2b_sb[:])
                nc.sync.dma_start(out[tt * P:(tt + 1) * P, :], yo[:])

    nc.compile()
    return nc


# =========================================================
# host side
# =========================================================
_CACHE = {}


def host_prep(cfg, inputs):
    """Full (unsharded) inputs -> list of per-core input maps."""
    E = cfg["E"]
    x = np.asarray(inputs["x"], np.float32)
    t = np.asarray(inputs["time"], np.float32)
    shared = dict(
        wqkvT=np.ascontiguousarray(
            np.asarray(inputs["w_qkv"], np.float32).T.astype(np.float16)),
        bqk=np.ascontiguousarray(
            np.asarray(inputs["b_qkv"], np.float32)[:2 * E, None]),
        bvrep=np.ascontiguousarray(
            np.tile(np.asarray(inputs["b_qkv"], np.float32)[None, 2 * E:], (P, 1))),
        woutT=np.ascontiguousarray(
            np.asarray(inputs["w_out"], np.float32).T.astype(np.float16)),
        bout=np.ascontiguousarray(np.asarray(inputs["b_out"], np.float32)[:, None]),
        ln1g=np.ascontiguousarray(np.asarray(inputs["ln1_g"], np.float32)[:, None]),
        ln1b=np.ascontiguousarray(np.asarray(inputs["ln1_b"], np.float32)[:, None]),
        ln2grep=np.ascontiguousarray(
            np.tile(np.asarray(inputs["ln2_g"], np.float32)[None, :], (P, 1))),
        ln2brep=np.ascontiguousarray(
            np.tile(np.asarray(inputs["ln2_b"], np.float32)[None, :], (P, 1))),
        gatew=np.ascontiguousarray(np.asarray(inputs["gate_w"], np.float32)),
        w1=np.ascontiguousarray(np.asarray(inputs["w1"]).astype(np.float16)),
        w2=np.ascontiguousarray(np.asarray(inputs["w2"]).astype(np.float16)),
    )
    in_maps = []
    for cid in range(cfg["NC"]):
        m = dict(shared)
        m["xT"] = np.ascontiguousarray(x[:, cid, :].T)
        m["tcol"] = np.ascontiguousarray(t[:, cid][:, None])
        m["trep"] = np.ascontiguousarray(np.tile(t[:, cid][None, :], (P, 1)))
        in_maps.append(m)
    return in_maps


def assemble(cfg, results):
    """Per-core 'out' (TOK, E) -> full (L, B, E)."""
    L, B, E, LC = cfg["L"], cfg["B"], cfg["E"], cfg["LC"]
    full = np.empty((L, B, E), np.float32)
    for cid in range(cfg["NC"]):
        o = np.asarray(results[cid]["out"]).reshape(LC, B, E)
        full[cid * LC:(cid + 1) * LC, :, :] = o
    return full


def get_built():
    if "full" not in _CACHE:
        cfg = make_cfg(FULL)
        _CACHE["full"] = (build_bass(cfg), cfg)
    return _CACHE["full"]


def kernel(**inputs):
    nc, cfg = get_built()
    in_maps = host_prep(cfg, inputs)
    res = run_bass_kernel_spmd(nc, in_maps, core_ids=list(range(cfg["NC"])))
    return assemble(cfg, res.results)


# revision 37
# speedup vs baseline: 1.7268x; 1.2301x over previous
"""Trainium2 Bass kernel: temporal-masked MHA + top2-gated MoE layer (8 NeuronCores).

Strategy (v2):
  - data-parallel attention over B (8 batches -> 8 cores), transposed layout,
    f16 matmul inputs (psum f32), block-causal skipping (time is sorted along
    L, so the temporal mask is block-causal; the diagonal blocks still use the
    real time comparison)
  - gate logits ride a separate tiny AllToAll so top-2 routing overlaps the
    main f16 y AllToAll
  - expert FFN: w1 weight-stationary -> hT, w2 activation-stationary (hts as
    lhsT) producing eo rows directly (no output transposes); big weight DMAs
    spread across engines; double-buffered psum
"""

import math
from contextlib import ExitStack

import numpy as np

import concourse.bass as bass
import concourse.bacc as bacc
import concourse.mybir as mybir
import concourse.tile as tile
from concourse.bass_utils import run_bass_kernel_spmd
from concourse.masks import make_identity

F32 = mybir.dt.float32
F32R = mybir.dt.float32r
F16 = mybir.dt.float16
I32 = mybir.dt.int32
AX = mybir.AxisListType
OP = mybir.AluOpType
ACT = mybir.ActivationFunctionType
P = 128

FULL = dict(L=512, B=8, E=1024, H=16, HID=4096, NE=5, NC=8)


def make_cfg(d):
    c = dict(d)
    c["CAP"] = max(min(c["B"], int(c["B"] * 2.0 / c["NE"])), 4)
    c["D"] = c["E"] // c["H"]
    assert c["D"] == 64, "head dim assumed 64"
    assert c["B"] == c["NC"]
    c["LC"] = c["L"] // c["NC"]          # L-groups per core
    c["TOK"] = c["LC"] * c["B"]          # MoE tokens per core
    assert c["TOK"] % P == 0
    assert c["L"] % P == 0
    assert c["NE"] <= 8
    c["GCAP"] = c["LC"] * c["CAP"]       # slots per expert per core
    return c


def tl(pool, shape, dtype=F32, *, tag, bufs=None):
    return pool.tile(list(shape), dtype, tag=tag, name=tag, bufs=bufs)


def build_bass(c):
    nc = bacc.Bacc("TRN2", target_bir_lowering=False, debug=False,
                   num_devices=c["NC"])
    L, B, E, H, HID, NE = c["L"], c["B"], c["E"], c["H"], c["HID"], c["NE"]
    CAP, LC, TOK, GCAP = c["CAP"], c["LC"], c["TOK"], c["GCAP"]
    KT = E // P                       # k-tiles over E
    MTOK = L // P                     # token tiles (attention, per batch)
    NTOKT = TOK // P                  # token tiles (MoE)
    NHT = 2 * E // P                  # qk row tiles
    HIDT = HID // P
    GPT = P // B                      # groups per 128-token tile
    spt = min(P, GCAP)                # slots per slot-tile
    nslt = (GCAP + P - 1) // P        # slot tiles per expert
    tpst = spt // (GPT * CAP)         # token-tiles per slot-tile
    sc = 1.0 / math.sqrt(64)

    # ---- I/O ----
    dt_ = nc.dram_tensor
    xT = dt_("xT", [E, L], F32, kind="ExternalInput")[:]
    tcol = dt_("tcol", [L, 1], F32, kind="ExternalInput")[:]
    trep = dt_("trep", [P, L], F32, kind="ExternalInput")[:]
    wqkvT = dt_("wqkvT", [E, 3 * E], F16, kind="ExternalInput")[:]
    bqk = dt_("bqk", [2 * E, 1], F32, kind="ExternalInput")[:]
    bvrep = dt_("bvrep", [P, E], F32, kind="ExternalInput")[:]
    woutT = dt_("woutT", [E, E], F16, kind="ExternalInput")[:]
    bout = dt_("bout", [E, 1], F32, kind="ExternalInput")[:]
    ln1g = dt_("ln1g", [E, 1], F32, kind="ExternalInput")[:]
    ln1b = dt_("ln1b", [E, 1], F32, kind="ExternalInput")[:]
    ln2g = dt_("ln2grep", [P, E], F32, kind="ExternalInput")[:]
    ln2b = dt_("ln2brep", [P, E], F32, kind="ExternalInput")[:]
    gw = dt_("gatew", [E, NE], F32, kind="ExternalInput")[:]   # pre-scaled by ln1_g
    gc0 = dt_("gatec0", [NE, 1], F32, kind="ExternalInput")[:]  # b @ gw
    gc1v = dt_("gatec1", [NE, 1], F32, kind="ExternalInput")[:]  # g @ gw
    w1 = dt_("w1", [NE, E, HID], F16, kind="ExternalInput")[:]
    w2 = dt_("w2", [NE, HID, E], F16, kind="ExternalInput")[:]
    out = dt_("out", [TOK, E], F32, kind="ExternalOutput")[:]

    # ---- host-side constant tables (baked into the NEFF) ----
    tri = np.zeros((P, P), np.float32)       # strict-lower within B-groups
    ob = np.zeros((P, P), np.float32)        # all-ones within B-groups
    for i in range(P):
        for j in range(P):
            if i // B == j // B:
                ob[i, j] = 1.0
                if i < j:
                    tri[i, j] = 1.0
    nsel = np.zeros((P, GPT), np.float32)
    for i in range(P):
        nsel[i, i // B] = float(i % B)
    iotac = np.tile(np.arange(CAP, dtype=np.float32), (P, 1))
    iotae = np.tile(np.arange(NE, dtype=np.float32), (P, 1))
    gbase = np.zeros((nslt, spt, 1), np.float32)
    for st in range(nslt):
        for p in range(spt):
            gbase[st, p, 0] = float(B * ((st * P + p) // CAP))
    gb2 = np.zeros((NTOKT, P, 1), np.float32)
    for t in range(NTOKT):
        for p in range(P):
            gb2[t, p, 0] = float(CAP * ((t * P + p) // B))
    # one-hot head-selector columns for the batched softmax denominator
    ehall = np.zeros((H, P, H), np.float16)
    for h in range(H):
        ehall[h, :, h] = 1.0
    # row-h selector/replicator: rep_h = selall[h].T @ recD  (row h -> 64 rows)
    selall = np.zeros((H, H, 64), np.float16)
    for h in range(H):
        selall[h, h, :] = 1.0

    # round-robin engines for weight-stream DMA triggering (vector cannot
    # trigger DMAs; gpsimd's collectives/gathers all precede these in
    # program order so its queue has slack during the FFN)
    dma_engines = [nc.sync, nc.scalar, nc.gpsimd]
    _ecnt = [0]

    def wdma(dst, src):
        e = dma_engines[_ecnt[0] % len(dma_engines)]
        _ecnt[0] += 1
        e.dma_start(dst, src)

    with tile.TileContext(nc) as tc, ExitStack() as ctx:
        cst = ctx.enter_context(tc.tile_pool(name="cst", bufs=1))
        dram = ctx.enter_context(tc.tile_pool(name="dram", bufs=1, space="DRAM"))
        pB = ctx.enter_context(tc.tile_pool(name="pB", bufs=1))

        def const_tile(arr, tag):
            ap = nc.inline_tensor(np.ascontiguousarray(arr), name=tag)[:]
            t = tl(cst, list(arr.shape), F32, tag=tag)
            nc.gpsimd.dma_start(t[:], ap)
            return t

        ident = tl(cst, [P, P], F32, tag="ident")
        make_identity(nc, ident[:])
        ident16 = tl(cst, [P, P], F16, tag="ident16")
        make_identity(nc, ident16[:])
        ones_t = tl(cst, [P, 1], F32, tag="ones")
        nc.vector.memset(ones_t[:], 1.0)
        onesr_t = tl(cst, [1, P], F32, tag="onesr")
        nc.vector.memset(onesr_t[:], 1.0)
        onesr16 = tl(cst, [1, P], F16, tag="onesr16")
        nc.vector.memset(onesr16[:], 1.0)
        ones16 = tl(cst, [P, 1], F16, tag="ones16")
        nc.vector.memset(ones16[:], 1.0)
        tri_t = const_tile(tri, "tri")
        ob_t = const_tile(ob, "ob")
        nsel_t = const_tile(nsel, "nsel")
        iotac_t = const_tile(iotac, "iotac")
        iotae_t = const_tile(iotae, "iotae")
        gbase_ap = nc.inline_tensor(gbase, name="gbase")[:]
        gbase_ts = []
        for st in range(nslt):
            t = tl(cst, [spt, 1], F32, tag=f"gbase{st}")
            nc.gpsimd.dma_start(t[:], gbase_ap[st])
            gbase_ts.append(t)
        gb2_ap = nc.inline_tensor(gb2, name="gb2")[:]
        gb2_ts = []
        for tt in range(NTOKT):
            t = tl(cst, [P, 1], F32, tag=f"gb2{tt}")
            nc.gpsimd.dma_start(t[:], gb2_ap[tt])
            gb2_ts.append(t)
        eh_ap = nc.inline_tensor(ehall, name="ehall")[:]
        eh_ts = []
        for h in range(H):
            t = tl(cst, [P, H], F16, tag=f"eh{h}")
            nc.gpsimd.dma_start(t[:], eh_ap[h])
            eh_ts.append(t)
        sel_ap = nc.inline_tensor(selall, name="selall")[:]
        sel_ts = []
        for h in range(H):
            t = tl(cst, [H, 64], F16, tag=f"sel{h}")
            nc.gpsimd.dma_start(t[:], sel_ap[h])
            sel_ts.append(t)
        gc0_t = tl(cst, [NE, 1], F32, tag="gc0")
        nc.gpsimd.dma_start(gc0_t[:], gc0)
        gc1_t = tl(cst, [NE, 1], F32, tag="gc1")
        nc.gpsimd.dma_start(gc1_t[:], gc1v)

        send16 = tl(dram, [L, E], F16, tag="send16")
        sendlg = tl(dram, [L, 8], F32, tag="sendlg")

        # =========================================================
        # PHASE A: attention for this core's batch (transposed layout, f16)
        # =========================================================
        y16 = []
        with tc.tile_pool(name="pA", bufs=1) as pA:
            xt = []
            xt16 = []
            for k in range(KT):
                t = tl(pA, [P, L], F32, tag=f"xt{k}")
                nc.gpsimd.dma_start(t[:], xT[k * P:(k + 1) * P, :])
                xt.append(t)
                t16 = tl(pA, [P, L], F16, tag=f"xt16_{k}")
                nc.vector.tensor_copy(t16[:], t[:])
                xt16.append(t16)
            bqk_t = tl(pA, [P, NHT], F32, tag="bqk")
            nc.gpsimd.dma_start(bqk_t[:], bqk.rearrange("(m p) o -> p (m o)", p=P))
            bv_t = []
            for nn in range(2):
                t = tl(pA, [P, 512], F32, tag=f"bv{nn}")
                nc.gpsimd.dma_start(t[:], bvrep[:, nn * 512:(nn + 1) * 512])
                bv_t.append(t)
            tcol_t = tl(pA, [P, MTOK], F32, tag="tcol")
            nc.gpsimd.dma_start(tcol_t[:], tcol.rearrange("(m p) o -> p (m o)", p=P))
            trep_t = tl(pA, [P, L], F32, tag="trep")
            nc.gpsimd.dma_start(trep_t[:], trep)
            gw_t = tl(pA, [P, KT * NE], F32, tag="gw")
            nc.sync.dma_start(gw_t[:].rearrange("p (k e) -> p k e", e=NE),
                              gw.rearrange("(k p) e -> p k e", p=P))
            bout_t = tl(pA, [P, KT], F32, tag="bout")
            nc.gpsimd.dma_start(bout_t[:], bout.rearrange("(m p) o -> p (m o)", p=P))
            ln1g_t = tl(pA, [P, KT], F32, tag="ln1g")
            nc.gpsimd.dma_start(ln1g_t[:], ln1g.rearrange("(m p) o -> p (m o)", p=P))
            ln1b_t = tl(pA, [P, KT], F32, tag="ln1b")
            nc.gpsimd.dma_start(ln1b_t[:], ln1b.rearrange("(m p) o -> p (m o)", p=P))

            # ---- qkT = wqkv[:2E] @ x^T (f16); V token-major w/ ones col
            qk = [tl(pA, [P, L], F16, tag=f"qk{m}") for m in range(NHT)]
            vt = [tl(pA, [P, E], F16, tag=f"vt{m}") for m in range(MTOK)]
            with tc.tile_pool(name="wp", bufs=2) as wp, \
                 tc.tile_pool(name="psQ", bufs=2, space="PSUM") as psQ:
                for mg in range(NHT // 4):
                    wq = []
                    for k in range(KT):
                        t = tl(wp, [P, 512], F16, tag=f"wq{k}")
                        nc.sync.dma_start(
                            t[:], wqkvT[k * P:(k + 1) * P,
                                        mg * 512:(mg + 1) * 512])
                        wq.append(t)
                    for j in range(4):
                        m = mg * 4 + j
                        ps = tl(psQ, [P, L], F32, tag="ps")
                        for k in range(KT):
                            nc.tensor.matmul(
                                ps[:], lhsT=wq[k][:, j * P:(j + 1) * P],
                                rhs=xt16[k][:], start=(k == 0),
                                stop=(k == KT - 1))
                        nc.vector.tensor_scalar_add(qk[m][:], ps[:],
                                                    bqk_t[:, m:m + 1])
                # V: x-stationary, weight cols streamed
                wv = {}
                for nn in range(2):
                    for k in range(KT):
                        t = tl(wp, [P, 512], F16, tag=f"wv{nn}_{k}", bufs=1)
                        nc.sync.dma_start(
                            t[:], wqkvT[k * P:(k + 1) * P,
                                        2 * E + nn * 512:2 * E + (nn + 1) * 512])
                        wv[(nn, k)] = t
                for mt in range(MTOK):
                    for nn in range(2):
                        ps = tl(psQ, [P, 512], F32, tag="ps")
                        for k in range(KT):
                            nc.tensor.matmul(
                                ps[:], lhsT=xt16[k][:, mt * P:(mt + 1) * P],
                                rhs=wv[(nn, k)][:], start=(k == 0),
                                stop=(k == KT - 1))
                        nc.vector.tensor_add(
                            vt[mt][:, nn * 512:(nn + 1) * 512], ps[:],
                            bv_t[nn][:])

            # ---- diagonal temporal masks (0 / -8e9; exp scale 1/8 -> -1e9)
            maskd = [tl(pA, [P, P], F32, tag=f"md{kt}") for kt in range(MTOK)]
            for kt in range(MTOK):
                nc.vector.tensor_tensor(
                    out=maskd[kt][:],
                    in0=tcol_t[:, kt:kt + 1].to_broadcast([P, P]),
                    in1=trep_t[:, kt * P:(kt + 1) * P], op=OP.is_gt)
                nc.vector.tensor_scalar_mul(maskd[kt][:], maskd[kt][:],
                                            -1e9 / sc)

            # ---- heads (block-causal: key tile kt only attends q >= kt*P)
            # denominators for ALL heads accumulate into one [H, L] psum via
            # one-hot lhsT columns; a single batched reciprocal replaces 16
            # serial [1,L] reciprocals (3.3us each)
            attnT = [tl(pA, [P, L], F16, tag=f"at{k}") for k in range(KT)]
            avS = [tl(pA, [64, L], F16, tag=f"avS{h}") for h in range(H)]
            with tc.tile_pool(name="pp", bufs=2) as pp, \
                 tc.tile_pool(name="smp", bufs=3) as smp, \
                 tc.tile_pool(name="psS", bufs=1, space="PSUM") as psS, \
                 tc.tile_pool(name="psD", bufs=1, space="PSUM") as psDp, \
                 tc.tile_pool(name="psAV", bufs=2, space="PSUM") as psAV:
                psD = tl(psDp, [H, L], F32, tag="psD")
                for h in range(H):
                    mq, rq = (h * 64) // P, (h * 64) % P
                    mk, rk = (E + h * 64) // P, (E + h * 64) % P
                    pts = []
                    for kt in range(MTOK):
                        N = L - kt * P
                        sps = tl(psS, [P, N], F32, tag=f"sps{kt}")
                        nc.tensor.matmul(
                            sps[:],
                            lhsT=qk[mk][rk:rk + 64, kt * P:(kt + 1) * P],
                            rhs=qk[mq][rq:rq + 64, kt * P:L],
                            start=True, stop=True)
                        nc.vector.tensor_add(sps[:, 0:P], sps[:, 0:P],
                                             maskd[kt][:])
                        pt_ = tl(pp, [P, N], F16, tag=f"pt{kt}")
                        nc.scalar.activation(pt_[:], sps[:], ACT.Exp, scale=sc)
                        pts.append(pt_)
                        nc.tensor.matmul(
                            psD[:, kt * P:L], lhsT=eh_ts[h][:], rhs=pt_[:],
                            start=(h == 0 and kt == 0),
                            stop=(h == H - 1 and kt == MTOK - 1),
                            skip_group_check=True)
                    av = tl(psAV, [64, L], F32, tag="av")
                    for qt in range(MTOK):
                        for kt in range(qt + 1):
                            nc.tensor.matmul(
                                av[:, qt * P:(qt + 1) * P],
                                lhsT=vt[kt][:, h * 64:h * 64 + 64],
                                rhs=pts[kt][:, (qt - kt) * P:(qt - kt + 1) * P],
                                start=(kt == 0), stop=(kt == qt))
                    nc.vector.tensor_copy(avS[h][:], av[:])
                # batched normalization
                recD = tl(smp, [H, L], F32, tag="recD")
                nc.vector.reciprocal(recD[:], psD[:])
                recD16 = tl(smp, [H, L], F16, tag="recD16")
                nc.vector.tensor_copy(recD16[:], recD[:])
                for h in range(H):
                    mq, rq = (h * 64) // P, (h * 64) % P
                    rep_ps = tl(psAV, [64, L], F32, tag="repps", bufs=1)
                    nc.tensor.matmul(rep_ps[:], lhsT=sel_ts[h][:],
                                     rhs=recD16[:], start=True,
                                     stop=True)
                    rep = tl(smp, [64, L], F16, tag="rep")
                    nc.vector.tensor_copy(rep[:], rep_ps[:])
                    nc.vector.tensor_mul(attnT[mq][rq:rq + 64, :], avS[h][:],
                                         rep[:])

            # ---- out-proj + residual (into xt -> zT)
            with tc.tile_pool(name="wp2", bufs=2) as wp2, \
                 tc.tile_pool(name="psO", bufs=2, space="PSUM") as psO:
                for mg in range(KT // 4):
                    wo = []
                    for k in range(KT):
                        t = tl(wp2, [P, 512], F16, tag=f"wo{k}")
                        nc.sync.dma_start(
                            t[:], woutT[k * P:(k + 1) * P,
                                        mg * 512:(mg + 1) * 512])
                        wo.append(t)
                    for j in range(4):
                        m = mg * 4 + j
                        ps = tl(psO, [P, L], F32, tag="ps")
                        for k in range(KT):
                            nc.tensor.matmul(ps[:],
                                             lhsT=wo[k][:, j * P:(j + 1) * P],
                                             rhs=attnT[k][:],
                                             start=(k == 0), stop=(k == KT - 1))
                        nc.vector.tensor_scalar_add(ps[:], ps[:],
                                                    bout_t[:, m:m + 1])
                        nc.vector.tensor_add(xt[m][:], ps[:], xt[m][:])  # zT

            # ---- LN1 stats + pre-LN gate logits
            # logits = rstd*(z @ (g.*gw)) - (rstd*mu)*(g@gw) + b@gw, so the
            # gate matmul runs on z directly and the logits A2A departs
            # before the y normalization loop
            for k in range(KT):
                y16.append(tl(pA, [P, L], F16, tag=f"y16_{k}"))
            with tc.tile_pool(name="lnp", bufs=3) as lnp, \
                 tc.tile_pool(name="gp", bufs=3) as gp, \
                 tc.tile_pool(name="psG", bufs=2, space="PSUM") as psG, \
                 tc.tile_pool(name="psL", bufs=1, space="PSUM") as psL:
                mu_ps = tl(psL, [1, L], F32, tag="mu")
                sq_ps = tl(psL, [1, L], F32, tag="sq")
                for k in range(KT):
                    z16 = tl(lnp, [P, L], F16, tag="z16")
                    nc.vector.tensor_copy(z16[:], xt[k][:])
                    nc.tensor.matmul(mu_ps[:], lhsT=ones16[:], rhs=z16[:],
                                     start=(k == 0), stop=(k == KT - 1))
                    sqt = tl(lnp, [P, L], F16, tag="sqt")
                    nc.scalar.activation(sqt[:], xt[k][:], ACT.Square)
                    nc.tensor.matmul(sq_ps[:], lhsT=ones16[:], rhs=sqt[:],
                                     start=(k == 0), stop=(k == KT - 1))
                gps = tl(psG, [NE, L], F32, tag="gps", bufs=1)
                for k in range(KT):
                    nc.tensor.matmul(
                        gps[:], lhsT=gw_t[:, k * NE:(k + 1) * NE],
                        rhs=xt[k][:], start=(k == 0), stop=(k == KT - 1))
                mu_r = tl(lnp, [1, L], F32, tag="mu_r")
                nc.vector.tensor_scalar_mul(mu_r[:], mu_ps[:], 1.0 / E)
                var_r = tl(lnp, [1, L], F32, tag="var_r")
                nc.vector.tensor_scalar_mul(var_r[:], sq_ps[:], 1.0 / E)
                mu2 = tl(lnp, [1, L], F32, tag="mu2")
                nc.vector.tensor_mul(mu2[:], mu_r[:], mu_r[:])
                nc.vector.tensor_sub(var_r[:], var_r[:], mu2[:])
                nc.vector.tensor_scalar_add(var_r[:], var_r[:], 1e-5)
                nc.scalar.sqrt(var_r[:], var_r[:])
                rstd_r = tl(lnp, [1, L], F32, tag="rstd_r")
                nc.vector.reciprocal(rstd_r[:], var_r[:])
                murst = tl(lnp, [1, L], F32, tag="murst")
                nc.vector.tensor_mul(murst[:], mu_r[:], rstd_r[:])
                mur16 = tl(lnp, [1, L], F16, tag="mur16")
                nc.vector.tensor_copy(mur16[:], mu_r[:])
                rstdr16 = tl(lnp, [1, L], F16, tag="rstdr16")
                nc.vector.tensor_copy(rstdr16[:], rstd_r[:])
                murst16 = tl(lnp, [1, L], F16, tag="murst16")
                nc.vector.tensor_copy(murst16[:], murst[:])
                # gate affine correction on [NE, L] (one psum bank, reused)
                rst5_ps = tl(psG, [NE, L], F32, tag="b5", bufs=1)
                nc.tensor.matmul(rst5_ps[:], lhsT=onesr16[:, 0:NE],
                                 rhs=rstdr16[:], start=True, stop=True)
                rst5 = tl(gp, [NE, L], F32, tag="rst5sb")
                nc.vector.tensor_copy(rst5[:], rst5_ps[:])
                mrst5_ps = tl(psG, [NE, L], F32, tag="b5", bufs=1)
                nc.tensor.matmul(mrst5_ps[:], lhsT=onesr16[:, 0:NE],
                                 rhs=murst16[:], start=True, stop=True)
                mrst5 = tl(gp, [NE, L], F32, tag="mrst5sb")
                nc.vector.tensor_scalar_mul(mrst5[:], mrst5_ps[:], gc1_t[:])
                lg_sb = tl(gp, [NE, L], F32, tag="lg_sb")
                nc.vector.tensor_tensor(out=lg_sb[:], in0=gps[:], in1=rst5[:],
                                        op=OP.mult)
                nc.vector.tensor_sub(lg_sb[:], lg_sb[:], mrst5[:])
                nc.vector.tensor_scalar_add(lg_sb[:], lg_sb[:], gc0_t[:])
                for ct in range(MTOK):
                    tp2 = tl(psG, [P, NE], F32, tag="tp2", bufs=1)
                    nc.tensor.transpose(tp2[:, 0:NE],
                                        lg_sb[:, ct * P:(ct + 1) * P],
                                        ident[0:NE, 0:NE])
                    lgr = tl(gp, [P, 8], F32, tag="lgr")
                    nc.vector.memset(lgr[:, NE:8], 0.0)
                    nc.vector.tensor_copy(lgr[:, 0:NE], tp2[:, 0:NE])
                    nc.sync.dma_start(sendlg[ct * P:(ct + 1) * P, :], lgr[:])

                # tiny logits AllToAll first: lets routing overlap the y A2A
                recvlg = tl(dram, [L, 8], F32, tag="recvlg")
                nc.gpsimd.collective_compute(
                    "AllToAll", OP.bypass,
                    replica_groups=[list(range(c["NC"]))],
                    ins=[sendlg[:].opt()], outs=[recvlg[:].opt()])

                # ---- y16 = LN1(z) (f16 direct; one psum bank reused)
                murep_ps = tl(psL, [P, L], F32, tag="brep", bufs=1)
                nc.tensor.matmul(murep_ps[:], lhsT=onesr16[:],
                                 rhs=mur16[:], start=True, stop=True)
                mu_rep = tl(lnp, [P, L], F32, tag="mu_rep")
                nc.vector.tensor_copy(mu_rep[:], murep_ps[:])
                rsrep_ps = tl(psL, [P, L], F32, tag="brep", bufs=1)
                nc.tensor.matmul(rsrep_ps[:], lhsT=onesr16[:],
                                 rhs=rstdr16[:], start=True, stop=True)
                rstd_rep = tl(lnp, [P, L], F32, tag="rstd_rep")
                nc.vector.tensor_copy(rstd_rep[:], rsrep_ps[:])
                for k in range(KT):
                    t1 = tl(lnp, [P, L], F32, tag="t1")
                    nc.vector.tensor_sub(t1[:], xt[k][:], mu_rep[:])
                    nc.vector.tensor_mul(t1[:], t1[:], rstd_rep[:])
                    nc.vector.tensor_scalar(
                        out=y16[k][:], in0=t1[:], scalar1=ln1g_t[:, k:k + 1],
                        scalar2=ln1b_t[:, k:k + 1], op0=OP.mult, op1=OP.add)

                # ---- y -> token-major f16 send buffer
                for ct in range(MTOK):
                    yrow = tl(gp, [P, E], F16, tag="yrow")
                    for k in range(KT):
                        tpY = tl(psG, [P, P], F16, tag="tpY")
                        nc.tensor.transpose(tpY[:], y16[k][:, ct * P:(ct + 1) * P],
                                            ident16[:])
                        nc.vector.tensor_copy(yrow[:, k * P:(k + 1) * P], tpY[:])
                    nc.sync.dma_start(send16[ct * P:(ct + 1) * P, :], yrow[:])

        # FFN weight pools open once attention SBUF is released; expert-0 w1
        # prefetch (tag per k, bufs=2: 8 tags x 2 x 4KB/part = 64KB/part)
        # streams during the A2A + routing window
        wf = ctx.enter_context(tc.tile_pool(name="wf", bufs=2))
        w2p = ctx.enter_context(tc.tile_pool(name="w2p", bufs=3))
        w1pre = {}
        for half in range(2):
            for k in range(KT):
                t = tl(wf, [P, 2048], F16, tag=f"w1_{k}")
                nc.sync.dma_start(
                    t[:], w1[0][k * P:(k + 1) * P, half * 2048:(half + 1) * 2048])
                w1pre[(half, k)] = t

        # =========================================================
        # AllToAll + permute to group-major token order
        # =========================================================
        recv16 = tl(dram, [L, E], F16, tag="recv16")
        nc.gpsimd.collective_compute(
            "AllToAll", OP.bypass,
            replica_groups=[list(range(c["NC"]))],
            ins=[send16[:].opt()], outs=[recv16[:].opt()])
        lgbuf = tl(dram, [TOK, 8], F32, tag="lgbuf")
        nc.sync.dma_start(
            lgbuf[:].rearrange("(l i) r -> l i r", i=c["NC"]),
            recvlg[:].rearrange("(i l) r -> l i r", i=c["NC"]))

        # =========================================================
        # PHASE B: top-2 routing with capacity (overlaps the y A2A)
        # =========================================================
        nmat_d = tl(dram, [NTOKT, GPT, NE * CAP], F32, tag="nmat_d")
        gc1 = [None] * NTOKT
        gc2 = [None] * NTOKT
        idx1 = [None] * NTOKT
        idx2 = [None] * NTOKT
        with tc.tile_pool(name="rt", bufs=2) as rt, \
             tc.tile_pool(name="psR", bufs=2, space="PSUM") as psR:
            for tt in range(NTOKT):
                lg = tl(rt, [P, NE], F32, tag="lg")
                nc.scalar.dma_start(lg[:], lgbuf[tt * P:(tt + 1) * P, 0:NE])
                mx = tl(rt, [P, 1], F32, tag="mx")
                nc.vector.reduce_max(mx[:], lg[:], axis=AX.X)
                nc.vector.tensor_scalar_mul(mx[:], mx[:], -1.0)
                ex = tl(rt, [P, NE], F32, tag="ex")
                sm = tl(rt, [P, 1], F32, tag="sm")
                nc.scalar.activation(ex[:], lg[:], ACT.Exp, bias=mx[:],
                                     accum_out=sm[:])
                rcp = tl(rt, [P, 1], F32, tag="rcp")
                nc.vector.reciprocal(rcp[:], sm[:])
                raw = tl(rt, [P, NE], F32, tag="raw")
                nc.vector.tensor_scalar_mul(raw[:], ex[:], rcp[:])

                def top1(rawt, tag):
                    g = tl(rt, [P, 1], F32, tag=f"g{tag}")
                    nc.vector.reduce_max(g[:], rawt[:], axis=AX.X)
                    eq = tl(rt, [P, NE], F32, tag=f"eq{tag}")
                    nc.vector.tensor_tensor(
                        out=eq[:], in0=rawt[:],
                        in1=g[:].to_broadcast([P, NE]), op=OP.is_ge)
                    cs = tl(rt, [P, NE], F32, tag=f"cs{tag}")
                    nc.vector.memset(cs[:, 0:1], 0.0)
                    for j in range(1, NE):
                        nc.vector.tensor_add(cs[:, j:j + 1], cs[:, j - 1:j],
                                             eq[:, j - 1:j])
                    fst = tl(rt, [P, NE], F32, tag=f"fst{tag}")
                    nc.vector.tensor_scalar(out=fst[:], in0=cs[:], scalar1=0.5,
                                            scalar2=None, op0=OP.is_lt)
                    m_ = tl(rt, [P, NE], F32, tag=f"m{tag}")
                    nc.vector.tensor_mul(m_[:], eq[:], fst[:])
                    return g, m_

                g1, m1r = top1(raw, "1")
                raw2 = tl(rt, [P, NE], F32, tag="raw2")
                nc.vector.tensor_mul(raw2[:], raw[:], m1r[:])
                nc.vector.tensor_sub(raw2[:], raw[:], raw2[:])
                g2, m2r = top1(raw2, "2")
                den = tl(rt, [P, 1], F32, tag="den")
                nc.vector.tensor_add(den[:], g1[:], g2[:])
                nc.vector.tensor_scalar_add(den[:], den[:], 1e-9)
                rd = tl(rt, [P, 1], F32, tag="rd")
                nc.vector.reciprocal(rd[:], den[:])
                g1n = tl(rt, [P, 1], F32, tag="g1n")
                nc.vector.tensor_mul(g1n[:], g1[:], rd[:])
                g2n = tl(rt, [P, 1], F32, tag="g2n")
                nc.vector.tensor_mul(g2n[:], g2[:], rd[:])

                pos1 = tl(psR, [P, NE], F32, tag="pos1")
                nc.tensor.matmul(pos1[:], lhsT=tri_t[:], rhs=m1r[:],
                                 start=True, stop=True)
                keep1 = tl(rt, [P, NE], F32, tag="keep1")
                nc.vector.tensor_scalar(out=keep1[:], in0=pos1[:],
                                        scalar1=CAP - 0.5, scalar2=None,
                                        op0=OP.is_lt)
                m1 = tl(rt, [P, NE], F32, tag="m1")
                nc.vector.tensor_mul(m1[:], m1r[:], keep1[:])
                pos2 = tl(psR, [P, NE], F32, tag="pos2")
                nc.tensor.matmul(pos2[:], lhsT=tri_t[:], rhs=m2r[:],
                                 start=True, stop=False)
                nc.tensor.matmul(pos2[:], lhsT=ob_t[:], rhs=m1[:],
                                 start=False, stop=True)
                keep2 = tl(rt, [P, NE], F32, tag="keep2")
                nc.vector.tensor_scalar(out=keep2[:], in0=pos2[:],
                                        scalar1=CAP - 0.5, scalar2=None,
                                        op0=OP.is_lt)
                m2 = tl(rt, [P, NE], F32, tag="m2")
                nc.vector.tensor_mul(m2[:], m2r[:], keep2[:])

                def dotX(a_ap, b_ap, tag):
                    t5 = tl(rt, [P, NE], F32, tag=f"t5{tag}")
                    nc.vector.tensor_mul(t5[:], a_ap, b_ap)
                    o = tl(rt, [P, 1], F32, tag=f"o{tag}")
                    nc.vector.reduce_sum(o[:], t5[:], axis=AX.X)
                    return o

                m1f = tl(rt, [P, 1], F32, tag="m1f")
                nc.vector.reduce_sum(m1f[:], m1[:], axis=AX.X)
                m2f = tl(rt, [P, 1], F32, tag="m2f")
                nc.vector.reduce_sum(m2f[:], m2[:], axis=AX.X)
                gc1[tt] = tl(pB, [P, 1], F32, tag=f"gc1_{tt}")
                nc.vector.tensor_mul(gc1[tt][:], g1n[:], m1f[:])
                gc2[tt] = tl(pB, [P, 1], F32, tag=f"gc2_{tt}")
                nc.vector.tensor_mul(gc2[tt][:], g2n[:], m2f[:])
                p1 = dotX(pos1[:], m1[:], "p1")
                p2 = dotX(pos2[:], m2[:], "p2")
                e1 = dotX(iotae_t[:], m1[:], "e1")
                e2 = dotX(iotae_t[:], m2[:], "e2")

                for (en, pn, sl) in ((e1, p1, 1), (e2, p2, 2)):
                    f_ = tl(rt, [P, 1], F32, tag=f"if{sl}")
                    nc.vector.tensor_scalar(
                        out=f_[:], in0=en[:], scalar1=float(GCAP),
                        scalar2=gb2_ts[tt][:], op0=OP.mult, op1=OP.add)
                    nc.vector.tensor_add(f_[:], f_[:], pn[:])
                    ix = tl(pB, [P, 1], I32, tag=f"idx{sl}_{tt}")
                    nc.vector.tensor_copy(ix[:], f_[:])
                    (idx1 if sl == 1 else idx2)[tt] = ix

                oh1 = tl(rt, [P, CAP], F32, tag="oh1")
                nc.vector.tensor_tensor(
                    out=oh1[:], in0=p1[:].to_broadcast([P, CAP]),
                    in1=iotac_t[:], op=OP.is_equal)
                oh2 = tl(rt, [P, CAP], F32, tag="oh2")
                nc.vector.tensor_tensor(
                    out=oh2[:], in0=p2[:].to_broadcast([P, CAP]),
                    in1=iotac_t[:], op=OP.is_equal)
                D = tl(rt, [P, NE * CAP], F32, tag="D")
                nc.vector.tensor_tensor(
                    out=D[:].rearrange("p (e c) -> p e c", c=CAP),
                    in0=m1[:].unsqueeze(2).to_broadcast([P, NE, CAP]),
                    in1=oh1[:].unsqueeze(1).to_broadcast([P, NE, CAP]),
                    op=OP.mult)
                D2 = tl(rt, [P, NE * CAP], F32, tag="D2")
                nc.vector.tensor_tensor(
                    out=D2[:].rearrange("p (e c) -> p e c", c=CAP),
                    in0=m2[:].unsqueeze(2).to_broadcast([P, NE, CAP]),
                    in1=oh2[:].unsqueeze(1).to_broadcast([P, NE, CAP]),
                    op=OP.mult)
                nc.vector.tensor_add(D[:], D[:], D2[:])
                nm = tl(psR, [GPT, NE * CAP], F32, tag="nm")
                nc.tensor.matmul(nm[:], lhsT=nsel_t[:], rhs=D[:],
                                 start=True, stop=True)
                nm_sb = tl(rt, [GPT, NE * CAP], F32, tag="nm_sb")
                nc.vector.tensor_copy(nm_sb[:], nm[:])
                nc.sync.dma_start(nmat_d[tt], nm_sb[:])

        # slot source-row indices (per expert / slot-tile), via DRAM relayout
        islot = [[None] * nslt for _ in range(NE)]
        with tc.tile_pool(name="ip", bufs=2) as ip:
            for e in range(NE):
                for st in range(nslt):
                    f_ = tl(ip, [spt, 1], F32, tag="f")
                    src = nmat_d[:][st * tpst:(st + 1) * tpst, :,
                                    e * CAP:(e + 1) * CAP]
                    nc.sync.dma_start(f_[:], src)
                    nc.vector.tensor_scalar_add(f_[:], f_[:], gbase_ts[st][:])
                    ix = tl(pB, [spt, 1], I32, tag=f"islot{e}_{st}")
                    nc.vector.tensor_copy(ix[:], f_[:])
                    islot[e][st] = ix

        # y A2A result -> group-major token order (issued here so the sync
        # queue is not blocked behind the big A2A during routing)
        ybuf16 = tl(dram, [TOK, E], F16, tag="ybuf16")
        nc.sync.dma_start(
            ybuf16[:].rearrange("(l i) r -> l i r", i=c["NC"]),
            recv16[:].rearrange("(i l) r -> l i r", i=c["NC"]))

        # =========================================================
        # expert FFN: gather -> einT -> w1/gelu -> w2 (act-stationary)
        # =========================================================
        eobuf = tl(dram, [NE * GCAP, E], F16, tag="eobuf")
        G1 = 2
        with tc.tile_pool(name="einp", bufs=1) as einp, \
             tc.tile_pool(name="eintp", bufs=1) as eintp, \
             tc.tile_pool(name="htp", bufs=2) as htp, \
             tc.tile_pool(name="eop", bufs=2) as eop, \
             tc.tile_pool(name="psF", bufs=1, space="PSUM") as psF, \
             tc.tile_pool(name="psW2", bufs=1, space="PSUM") as psW2, \
             tc.tile_pool(name="psT", bufs=2, space="PSUM") as psT:
            # prefetch ALL expert gathers up front (gpsimd queue)
            eins = {}
            for e in range(NE):
                for st in range(nslt):
                    g_ = tl(einp, [spt, E], F16, tag=f"g{e}_{st}")
                    nc.gpsimd.indirect_dma_start(
                        out=g_[:], out_offset=None, in_=ybuf16[:],
                        in_offset=bass.IndirectOffsetOnAxis(
                            ap=islot[e][st][:, :1], axis=0))
                    eins[(e, st)] = g_

            # fused per-expert pipeline: w2 matmuls for hid-tile pair (mg-1)
            # interleave with w1 matmuls for group mg, so gelu latency hides
            # and the PE never drains between the two GEMMs
            NMG = 32 // G1
            for e in range(NE):
                # einT for this expert (just-in-time on tensor queue)
                einT = {}
                for k in range(KT):
                    t_ = tl(eintp, [P, GCAP], F16, tag=f"einT{e}_{k}")
                    for st in range(nslt):
                        tp3 = tl(psT, [P, P], F16, tag="tp3")
                        nc.tensor.transpose(tp3[:, 0:spt],
                                            eins[(e, st)][:, k * P:(k + 1) * P],
                                            ident16[0:spt, 0:spt])
                        nc.vector.tensor_copy(t_[:, st * P:st * P + spt],
                                              tp3[:, 0:spt])
                    einT[k] = t_
                hts = {}
                pw = [tl(psW2, [P, 512], F32, tag=f"pw{i}")
                      for i in range(2 * nslt)]

                def w2_block(kh):
                    w2r = tl(w2p, [P, E], F16, tag="w2r")
                    wdma(w2r[:], w2[e][kh * P:(kh + 1) * P, :])
                    for sb in range(nslt):
                        for ch in range(2):
                            nc.tensor.matmul(
                                pw[sb * 2 + ch][:],
                                lhsT=hts[kh][:, sb * P:sb * P + spt],
                                rhs=w2r[:, ch * 512:(ch + 1) * 512],
                                start=(kh == 0), stop=(kh == HIDT - 1))

                w1t = {}
                for mg in range(NMG):
                    half, hmg = mg // 8, mg % 8
                    if hmg == 0:
                        for k in range(KT):
                            if e == 0:
                                w1t[k] = w1pre[(half, k)]
                            else:
                                t = tl(wf, [P, 2048], F16, tag=f"w1_{k}")
                                wdma(t[:], w1[e][k * P:(k + 1) * P,
                                                 half * 2048:(half + 1) * 2048])
                                w1t[k] = t
                    pss = [tl(psF, [P, GCAP], F32, tag=f"ps{j}")
                           for j in range(G1)]
                    for k in range(KT):
                        for j in range(G1):
                            col = (hmg * G1 + j) * P
                            nc.tensor.matmul(
                                pss[j][:], lhsT=w1t[k][:, col:col + P],
                                rhs=einT[k][:], start=(k == 0),
                                stop=(k == KT - 1))
                    for j in range(G1):
                        kh = mg * G1 + j
                        ht_ = tl(htp, [P, GCAP], F16, tag=f"ht{kh}")
                        nc.scalar.activation(ht_[:], pss[j][:], ACT.Gelu)
                        hts[kh] = ht_
                    if mg > 0:
                        for j in range(G1):
                            w2_block((mg - 1) * G1 + j)
                for j in range(G1):
                    w2_block((NMG - 1) * G1 + j)
                for sb in range(nslt):
                    for ch in range(2):
                        eo16 = tl(eop, [P, 512], F16, tag="eo16")
                        nc.vector.tensor_copy(eo16[0:spt, :],
                                              pw[sb * 2 + ch][0:spt, :])
                        nc.sync.dma_start(
                            eobuf[e * GCAP + sb * P:e * GCAP + sb * P + spt,
                                  ch * 512:(ch + 1) * 512], eo16[0:spt, :])

        # =========================================================
        # combine + LN2 -> out
        # =========================================================
        with tc.tile_pool(name="cb", bufs=2) as cb:
            ln2g_sb = tl(pB, [P, E], F32, tag="ln2g")
            nc.gpsimd.dma_start(ln2g_sb[:], ln2g)
            ln2b_sb = tl(pB, [P, E], F32, tag="ln2b")
            nc.gpsimd.dma_start(ln2b_sb[:], ln2b)
            for tt in range(NTOKT):
                o1 = tl(cb, [P, E], F16, tag="o1")
                nc.gpsimd.indirect_dma_start(
                    out=o1[:], out_offset=None, in_=eobuf[:],
                    in_offset=bass.IndirectOffsetOnAxis(ap=idx1[tt][:, :1], axis=0))
                o2 = tl(cb, [P, E], F16, tag="o2")
                nc.gpsimd.indirect_dma_start(
                    out=o2[:], out_offset=None, in_=eobuf[:],
                    in_offset=bass.IndirectOffsetOnAxis(ap=idx2[tt][:, :1], axis=0))
                ysb = tl(cb, [P, E], F16, tag="ysb")
                nc.sync.dma_start(ysb[:], ybuf16[tt * P:(tt + 1) * P, :])
                # f16->f32 conversion + per-token scaling on the scalar
                # engine; all-f32 vector arithmetic (mixed-dtype DVE ops are
                # ~5x slower)
                s1 = tl(cb, [P, E], F32, tag="s1")
                nc.scalar.activation(s1[:], o1[:], ACT.Copy, scale=gc1[tt][:])
                s2 = tl(cb, [P, E], F32, tag="s2")
                nc.scalar.activation(s2[:], o2[:], ACT.Copy, scale=gc2[tt][:])
                ysbf = tl(cb, [P, E], F32, tag="ysbf")
                nc.scalar.copy(ysbf[:], ysb[:])
                z = tl(cb, [P, E], F32, tag="z")
                nc.vector.tensor_add(z[:], s1[:], s2[:])
                nc.vector.tensor_add(z[:], z[:], ysbf[:])
                mu = tl(cb, [P, 1], F32, tag="mu")
                nc.vector.reduce_sum(mu[:], z[:], axis=AX.X)
                nc.vector.tensor_scalar_mul(mu[:], mu[:], 1.0 / E)
                xc = tl(cb, [P, E], F32, tag="xc")
                nc.vector.tensor_scalar(out=xc[:], in0=z[:], scalar1=mu[:],
                                        scalar2=None, op0=OP.subtract)
                scr = tl(cb, [P, E], F32, tag="scr")
                ssq = tl(cb, [P, 1], F32, tag="ssq")
                nc.scalar.activation(scr[:], xc[:], ACT.Square, accum_out=ssq[:])
                nc.vector.tensor_scalar(out=ssq[:], in0=ssq[:], scalar1=1.0 / E,
                                        scalar2=1e-5, op0=OP.mult, op1=OP.add)
                nc.scalar.sqrt(ssq[:], ssq[:])
                rstd = tl(cb, [P, 1], F32, tag="rstd")
                nc.vector.reciprocal(rstd[:], ssq[:])
                nc.vector.tensor_scalar_mul(xc[:], xc[:], rstd[:])
                yo = tl(cb, [P, E], F32, tag="yo")
                nc.vector.tensor_mul(yo[:], xc[:], ln2g_sb[:])
                nc.vector.tensor_add(yo[:], yo[:], ln2b_sb[:])
                nc.sync.dma_start(out[tt * P:(tt + 1) * P, :], yo[:])

    nc.compile()
    return nc


# =========================================================
# host side
# =========================================================
_CACHE = {}


def host_prep(cfg, inputs):
    """Full (unsharded) inputs -> list of per-core input maps."""
    E = cfg["E"]
    x = np.asarray(inputs["x"], np.float32)
    t = np.asarray(inputs["time"], np.float32)
    shared = dict(
        wqkvT=np.ascontiguousarray(
            np.asarray(inputs["w_qkv"], np.float32).T.astype(np.float16)),
        bqk=np.ascontiguousarray(
            np.asarray(inputs["b_qkv"], np.float32)[:2 * E, None]),
        bvrep=np.ascontiguousarray(
            np.tile(np.asarray(inputs["b_qkv"], np.float32)[None, 2 * E:], (P, 1))),
        woutT=np.ascontiguousarray(
            np.asarray(inputs["w_out"], np.float32).T.astype(np.float16)),
        bout=np.ascontiguousarray(np.asarray(inputs["b_out"], np.float32)[:, None]),
        ln1g=np.ascontiguousarray(np.asarray(inputs["ln1_g"], np.float32)[:, None]),
        ln1b=np.ascontiguousarray(np.asarray(inputs["ln1_b"], np.float32)[:, None]),
        ln2grep=np.ascontiguousarray(
            np.tile(np.asarray(inputs["ln2_g"], np.float32)[None, :], (P, 1))),
        ln2brep=np.ascontiguousarray(
            np.tile(np.asarray(inputs["ln2_b"], np.float32)[None, :], (P, 1))),
        gatew=np.ascontiguousarray(
            np.asarray(inputs["ln1_g"], np.float32)[:, None]
            * np.asarray(inputs["gate_w"], np.float32)),
        gatec0=np.ascontiguousarray(
            (np.asarray(inputs["ln1_b"], np.float32)
             @ np.asarray(inputs["gate_w"], np.float32))[:, None]),
        gatec1=np.ascontiguousarray(
            (np.asarray(inputs["ln1_g"], np.float32)
             @ np.asarray(inputs["gate_w"], np.float32))[:, None]),
        w1=np.ascontiguousarray(np.asarray(inputs["w1"]).astype(np.float16)),
        w2=np.ascontiguousarray(np.asarray(inputs["w2"]).astype(np.float16)),
    )
    in_maps = []
    for cid in range(cfg["NC"]):
        m = dict(shared)
        m["xT"] = np.ascontiguousarray(x[:, cid, :].T)
        m["tcol"] = np.ascontiguousarray(t[:, cid][:, None])
        m["trep"] = np.ascontiguousarray(np.tile(t[:, cid][None, :], (P, 1)))
        in_maps.append(m)
    return in_maps


def assemble(cfg, results):
    """Per-core 'out' (TOK, E) -> full (L, B, E)."""
    L, B, E, LC = cfg["L"], cfg["B"], cfg["E"], cfg["LC"]
    full = np.empty((L, B, E), np.float32)
    for cid in range(cfg["NC"]):
        o = np.asarray(results[cid]["out"]).reshape(LC, B, E)
        full[cid * LC:(cid + 1) * LC, :, :] = o
    return full


def get_built():
    if "full" not in _CACHE:
        cfg = make_cfg(FULL)
        _CACHE["full"] = (build_bass(cfg), cfg)
    return _CACHE["full"]


def kernel(**inputs):
    nc, cfg = get_built()
    in_maps = host_prep(cfg, inputs)
    res = run_bass_kernel_spmd(nc, in_maps, core_ids=list(range(cfg["NC"])))
    return assemble(cfg, res.results)


# revision 49
# speedup vs baseline: 1.7501x; 1.0135x over previous
"""Trainium2 Bass kernel: temporal-masked MHA + top2-gated MoE layer (8 NeuronCores).

Strategy (v2):
  - data-parallel attention over B (8 batches -> 8 cores), transposed layout,
    f16 matmul inputs (psum f32), block-causal skipping (time is sorted along
    L, so the temporal mask is block-causal; the diagonal blocks still use the
    real time comparison)
  - gate logits ride a separate tiny AllToAll so top-2 routing overlaps the
    main f16 y AllToAll
  - expert FFN: w1 weight-stationary -> hT, w2 activation-stationary (hts as
    lhsT) producing eo rows directly (no output transposes); big weight DMAs
    spread across engines; double-buffered psum
"""

import math
from contextlib import ExitStack

import numpy as np

import concourse.bass as bass
import concourse.bacc as bacc
import concourse.mybir as mybir
import concourse.tile as tile
from concourse.bass_utils import run_bass_kernel_spmd
from concourse.masks import make_identity

F32 = mybir.dt.float32
F32R = mybir.dt.float32r
F16 = mybir.dt.float16
I32 = mybir.dt.int32
AX = mybir.AxisListType
OP = mybir.AluOpType
ACT = mybir.ActivationFunctionType
P = 128

FULL = dict(L=512, B=8, E=1024, H=16, HID=4096, NE=5, NC=8)


def make_cfg(d):
    c = dict(d)
    c["CAP"] = max(min(c["B"], int(c["B"] * 2.0 / c["NE"])), 4)
    c["D"] = c["E"] // c["H"]
    assert c["D"] == 64, "head dim assumed 64"
    assert c["B"] == c["NC"]
    c["LC"] = c["L"] // c["NC"]          # L-groups per core
    c["TOK"] = c["LC"] * c["B"]          # MoE tokens per core
    assert c["TOK"] % P == 0
    assert c["L"] % P == 0
    assert c["NE"] <= 8
    c["GCAP"] = c["LC"] * c["CAP"]       # slots per expert per core
    return c


def tl(pool, shape, dtype=F32, *, tag, bufs=None):
    return pool.tile(list(shape), dtype, tag=tag, name=tag, bufs=bufs)


def build_bass(c):
    nc = bacc.Bacc("TRN2", target_bir_lowering=False, debug=False,
                   num_devices=c["NC"])
    L, B, E, H, HID, NE = c["L"], c["B"], c["E"], c["H"], c["HID"], c["NE"]
    CAP, LC, TOK, GCAP = c["CAP"], c["LC"], c["TOK"], c["GCAP"]
    KT = E // P                       # k-tiles over E
    MTOK = L // P                     # token tiles (attention, per batch)
    NTOKT = TOK // P                  # token tiles (MoE)
    NHT = 2 * E // P                  # qk row tiles
    HIDT = HID // P
    GPT = P // B                      # groups per 128-token tile
    spt = min(P, GCAP)                # slots per slot-tile
    nslt = (GCAP + P - 1) // P        # slot tiles per expert
    tpst = spt // (GPT * CAP)         # token-tiles per slot-tile
    sc = 1.0 / math.sqrt(64)

    # ---- I/O ----
    dt_ = nc.dram_tensor
    xT = dt_("xT", [E, L], F32, kind="ExternalInput")[:]
    tcol = dt_("tcol", [L, 1], F32, kind="ExternalInput")[:]
    trep = dt_("trep", [P, L], F32, kind="ExternalInput")[:]
    wqkvT = dt_("wqkvT", [E, 3 * E], F16, kind="ExternalInput")[:]
    bqk = dt_("bqk", [2 * E, 1], F32, kind="ExternalInput")[:]
    bvrep = dt_("bvrep", [P, E], F32, kind="ExternalInput")[:]
    woutT = dt_("woutT", [E, E], F16, kind="ExternalInput")[:]
    bout = dt_("bout", [E, 1], F32, kind="ExternalInput")[:]
    ln1g = dt_("ln1g", [E, 1], F32, kind="ExternalInput")[:]
    ln1b = dt_("ln1b", [E, 1], F32, kind="ExternalInput")[:]
    ln2g = dt_("ln2grep", [P, E], F32, kind="ExternalInput")[:]
    ln2b = dt_("ln2brep", [P, E], F32, kind="ExternalInput")[:]
    gw = dt_("gatew", [E, NE], F32, kind="ExternalInput")[:]   # pre-scaled by ln1_g
    gc0 = dt_("gatec0", [NE, 1], F32, kind="ExternalInput")[:]  # b @ gw
    gc1v = dt_("gatec1", [NE, 1], F32, kind="ExternalInput")[:]  # g @ gw
    w1 = dt_("w1", [NE, E, HID], F16, kind="ExternalInput")[:]
    w2 = dt_("w2", [NE, HID, E], F16, kind="ExternalInput")[:]
    out = dt_("out", [TOK, E], F32, kind="ExternalOutput")[:]

    # ---- host-side constant tables (baked into the NEFF) ----
    tri = np.zeros((P, P), np.float32)       # strict-lower within B-groups
    ob = np.zeros((P, P), np.float32)        # all-ones within B-groups
    for i in range(P):
        for j in range(P):
            if i // B == j // B:
                ob[i, j] = 1.0
                if i < j:
                    tri[i, j] = 1.0
    nsel = np.zeros((P, GPT), np.float32)
    for i in range(P):
        nsel[i, i // B] = float(i % B)
    iotac4 = np.tile(np.arange(CAP, dtype=np.float32), (P, NTOKT))
    iotae4 = np.tile(np.arange(NE, dtype=np.float32), (P, NTOKT))
    gbase = np.zeros((nslt, spt, 1), np.float32)
    for st in range(nslt):
        for p in range(spt):
            gbase[st, p, 0] = float(B * ((st * P + p) // CAP))
    gb2c = np.zeros((P, NTOKT), np.float32)
    for t in range(NTOKT):
        for p in range(P):
            gb2c[p, t] = float(CAP * ((t * P + p) // B))
    egcap = np.tile(np.repeat(np.arange(NE, dtype=np.float32) * GCAP, 1),
                    (P, NTOKT))  # [P, NTOKT*NE]: e*GCAP per (t,e) column
    # one-hot head-selector columns for the batched softmax denominator
    ehall = np.zeros((P, H * H), np.float16)
    for h in range(H):
        ehall[:, h * H + h] = 1.0
    # row-h selector/replicator: rep_h = sel[h].T @ recD  (row h -> 64 rows)
    selall = np.zeros((H, H * 64), np.float16)
    for h in range(H):
        selall[h, h * 64:(h + 1) * 64] = 1.0

    # round-robin engines for weight-stream DMA triggering (vector cannot
    # trigger DMAs; gpsimd's collectives/gathers all precede these in
    # program order so its queue has slack during the FFN)
    dma_engines = [nc.sync, nc.scalar, nc.gpsimd]
    _ecnt = [0]

    def wdma(dst, src):
        e = dma_engines[_ecnt[0] % len(dma_engines)]
        _ecnt[0] += 1
        e.dma_start(dst, src)

    with tile.TileContext(nc) as tc, ExitStack() as ctx:
        cst = ctx.enter_context(tc.tile_pool(name="cst", bufs=1))
        dram = ctx.enter_context(tc.tile_pool(name="dram", bufs=1, space="DRAM"))
        pB = ctx.enter_context(tc.tile_pool(name="pB", bufs=1))
        # pB is the bottom of the SBUF stack and must not grow after later
        # pools stack above it -- allocate every persistent tile up front
        ln2g_sb = tl(pB, [P, E], F32, tag="ln2g")
        ln2b_sb = tl(pB, [P, E], F32, tag="ln2b")
        acc = [tl(pB, [P, E], F32, tag=f"acc{tt}") for tt in range(NTOKT)]
        gsel = tl(pB, [P, NTOKT * NE], F32, tag="gsel")
        gca = tl(pB, [P, NTOKT], F32, tag="gca")
        gcb = tl(pB, [P, NTOKT], F32, tag="gcb")
        idxsel = {}
        for e_ in range(NE):
            for tt in range(NTOKT):
                idxsel[(e_, tt)] = tl(pB, [P, 1], I32, tag=f"ix{e_}_{tt}")
        islot = [[tl(pB, [spt, 1], I32, tag=f"islot{e_}_{st}")
                  for st in range(nslt)] for e_ in range(NE)]

        def const_tile(arr, tag):
            ap = nc.inline_tensor(np.ascontiguousarray(arr), name=tag)[:]
            t = tl(cst, list(arr.shape), F32, tag=tag)
            nc.gpsimd.dma_start(t[:], ap)
            return t

        ident = tl(cst, [P, P], F32, tag="ident")
        make_identity(nc, ident[:])
        ident16 = tl(cst, [P, P], F16, tag="ident16")
        make_identity(nc, ident16[:])
        ones_t = tl(cst, [P, 1], F32, tag="ones")
        nc.vector.memset(ones_t[:], 1.0)
        onesr_t = tl(cst, [1, P], F32, tag="onesr")
        nc.vector.memset(onesr_t[:], 1.0)
        onesr16 = tl(cst, [1, P], F16, tag="onesr16")
        nc.vector.memset(onesr16[:], 1.0)
        ones16 = tl(cst, [P, 1], F16, tag="ones16")
        nc.vector.memset(ones16[:], 1.0)
        tri_t = const_tile(tri, "tri")
        ob_t = const_tile(ob, "ob")
        nsel_t = const_tile(nsel, "nsel")
        iotac_t = const_tile(iotac4, "iotac4")
        iotae_t = const_tile(iotae4, "iotae4")
        gb2_t = const_tile(gb2c, "gb2c")
        egcap_t = const_tile(egcap, "egcap")
        gbase_ap = nc.inline_tensor(gbase, name="gbase")[:]
        gbase_ts = []
        for st in range(nslt):
            t = tl(cst, [spt, 1], F32, tag=f"gbase{st}")
            nc.gpsimd.dma_start(t[:], gbase_ap[st])
            gbase_ts.append(t)
        ehbig = nc.inline_tensor(ehall, name="ehall")[:]
        ehb_t = tl(cst, [P, H * H], F16, tag="ehbig")
        nc.gpsimd.dma_start(ehb_t[:], ehbig)
        eh_ts = [ehb_t[:, h * H:(h + 1) * H] for h in range(H)]
        selbig = nc.inline_tensor(selall, name="selall")[:]
        selb_t = tl(cst, [H, H * 64], F16, tag="selbig")
        nc.gpsimd.dma_start(selb_t[:], selbig)
        sel_ts = [selb_t[:, h * 64:(h + 1) * 64] for h in range(H)]
        gc0_t = tl(cst, [NE, 1], F32, tag="gc0")
        nc.gpsimd.dma_start(gc0_t[:], gc0)
        gc1_t = tl(cst, [NE, 1], F32, tag="gc1")
        nc.gpsimd.dma_start(gc1_t[:], gc1v)

        send16 = tl(dram, [L, E], F16, tag="send16")
        sendlg = tl(dram, [L, 8], F32, tag="sendlg")

        # =========================================================
        # PHASE A: attention for this core's batch (transposed layout, f16)
        # =========================================================
        y16 = []
        with tc.tile_pool(name="pA", bufs=1) as pA:
            # x loads lead the sync queue; small operands ride scalar so
            # the const stream on gpsimd never gates the critical path
            xt = []
            xt16 = []
            for k in range(KT):
                t = tl(pA, [P, L], F32, tag=f"xt{k}")
                nc.sync.dma_start(t[:], xT[k * P:(k + 1) * P, :])
                xt.append(t)
                t16 = tl(pA, [P, L], F16, tag=f"xt16_{k}")
                nc.vector.tensor_copy(t16[:], t[:])
                xt16.append(t16)
            bqk_t = tl(pA, [P, NHT], F32, tag="bqk")
            nc.scalar.dma_start(bqk_t[:], bqk.rearrange("(m p) o -> p (m o)", p=P))
            bv_t = []
            for nn in range(2):
                t = tl(pA, [P, 512], F32, tag=f"bv{nn}")
                nc.scalar.dma_start(t[:], bvrep[:, nn * 512:(nn + 1) * 512])
                bv_t.append(t)
            tcol_t = tl(pA, [P, MTOK], F32, tag="tcol")
            nc.scalar.dma_start(tcol_t[:], tcol.rearrange("(m p) o -> p (m o)", p=P))
            trep_t = tl(pA, [P, L], F32, tag="trep")
            nc.scalar.dma_start(trep_t[:], trep)
            gw_t = tl(pA, [P, KT * NE], F32, tag="gw")
            nc.scalar.dma_start(gw_t[:].rearrange("p (k e) -> p k e", e=NE),
                                gw.rearrange("(k p) e -> p k e", p=P))
            bout_t = tl(pA, [P, KT], F32, tag="bout")
            nc.scalar.dma_start(bout_t[:], bout.rearrange("(m p) o -> p (m o)", p=P))
            ln1g_t = tl(pA, [P, KT], F32, tag="ln1g")
            nc.scalar.dma_start(ln1g_t[:], ln1g.rearrange("(m p) o -> p (m o)", p=P))
            ln1b_t = tl(pA, [P, KT], F32, tag="ln1b")
            nc.scalar.dma_start(ln1b_t[:], ln1b.rearrange("(m p) o -> p (m o)", p=P))

            # ---- qkT = wqkv[:2E] @ x^T (f16); V token-major w/ ones col
            qk = [tl(pA, [P, L], F16, tag=f"qk{m}") for m in range(NHT)]
            vt = [tl(pA, [P, E], F16, tag=f"vt{m}") for m in range(MTOK)]
            with tc.tile_pool(name="wp", bufs=2) as wp, \
                 tc.tile_pool(name="psQ", bufs=2, space="PSUM") as psQ:
                for mg in range(NHT // 4):
                    wq = []
                    for k in range(KT):
                        t = tl(wp, [P, 512], F16, tag=f"wq{k}")
                        nc.sync.dma_start(
                            t[:], wqkvT[k * P:(k + 1) * P,
                                        mg * 512:(mg + 1) * 512])
                        wq.append(t)
                    for j in range(4):
                        m = mg * 4 + j
                        ps = tl(psQ, [P, L], F32, tag="ps")
                        for k in range(KT):
                            nc.tensor.matmul(
                                ps[:], lhsT=wq[k][:, j * P:(j + 1) * P],
                                rhs=xt16[k][:], start=(k == 0),
                                stop=(k == KT - 1))
                        nc.vector.tensor_scalar_add(qk[m][:], ps[:],
                                                    bqk_t[:, m:m + 1])
                # V: x-stationary, weight cols streamed
                wv = {}
                for nn in range(2):
                    for k in range(KT):
                        t = tl(wp, [P, 512], F16, tag=f"wv{nn}_{k}", bufs=1)
                        nc.sync.dma_start(
                            t[:], wqkvT[k * P:(k + 1) * P,
                                        2 * E + nn * 512:2 * E + (nn + 1) * 512])
                        wv[(nn, k)] = t
                for mt in range(MTOK):
                    for nn in range(2):
                        ps = tl(psQ, [P, 512], F32, tag="ps")
                        for k in range(KT):
                            nc.tensor.matmul(
                                ps[:], lhsT=xt16[k][:, mt * P:(mt + 1) * P],
                                rhs=wv[(nn, k)][:], start=(k == 0),
                                stop=(k == KT - 1))
                        nc.vector.tensor_add(
                            vt[mt][:, nn * 512:(nn + 1) * 512], ps[:],
                            bv_t[nn][:])

            # ---- diagonal temporal masks (0 / -8e9; exp scale 1/8 -> -1e9)
            maskd = [tl(pA, [P, P], F32, tag=f"md{kt}") for kt in range(MTOK)]
            for kt in range(MTOK):
                nc.vector.tensor_tensor(
                    out=maskd[kt][:],
                    in0=tcol_t[:, kt:kt + 1].to_broadcast([P, P]),
                    in1=trep_t[:, kt * P:(kt + 1) * P], op=OP.is_gt)
                nc.vector.tensor_scalar_mul(maskd[kt][:], maskd[kt][:],
                                            -1e9 / sc)

            # ---- heads (block-causal: key tile kt only attends q >= kt*P)
            # denominators for ALL heads accumulate into one [H, L] psum via
            # one-hot lhsT columns; a single batched reciprocal replaces 16
            # serial [1,L] reciprocals (3.3us each)
            attnT = [tl(pA, [P, L], F16, tag=f"at{k}") for k in range(KT)]
            avS = [tl(pA, [64, L], F16, tag=f"avS{h}") for h in range(H)]
            with tc.tile_pool(name="pp", bufs=2) as pp, \
                 tc.tile_pool(name="smp", bufs=3) as smp, \
                 tc.tile_pool(name="psS", bufs=1, space="PSUM") as psS, \
                 tc.tile_pool(name="psD", bufs=1, space="PSUM") as psDp, \
                 tc.tile_pool(name="psAV", bufs=2, space="PSUM") as psAV:
                psD = tl(psDp, [H, L], F32, tag="psD")
                for h in range(H):
                    mq, rq = (h * 64) // P, (h * 64) % P
                    mk, rk = (E + h * 64) // P, (E + h * 64) % P
                    pts = []
                    for kt in range(MTOK):
                        N = L - kt * P
                        sps = tl(psS, [P, N], F32, tag=f"sps{kt}")
                        nc.tensor.matmul(
                            sps[:],
                            lhsT=qk[mk][rk:rk + 64, kt * P:(kt + 1) * P],
                            rhs=qk[mq][rq:rq + 64, kt * P:L],
                            start=True, stop=True)
                        nc.vector.tensor_add(sps[:, 0:P], sps[:, 0:P],
                                             maskd[kt][:])
                        pt_ = tl(pp, [P, N], F16, tag=f"pt{kt}")
                        nc.scalar.activation(pt_[:], sps[:], ACT.Exp, scale=sc)
                        pts.append(pt_)
                        nc.tensor.matmul(
                            psD[:, kt * P:L], lhsT=eh_ts[h], rhs=pt_[:],
                            start=(h == 0 and kt == 0),
                            stop=(h == H - 1 and kt == MTOK - 1),
                            skip_group_check=True)
                    av = tl(psAV, [64, L], F32, tag="av")
                    for qt in range(MTOK):
                        for kt in range(qt + 1):
                            nc.tensor.matmul(
                                av[:, qt * P:(qt + 1) * P],
                                lhsT=vt[kt][:, h * 64:h * 64 + 64],
                                rhs=pts[kt][:, (qt - kt) * P:(qt - kt + 1) * P],
                                start=(kt == 0), stop=(kt == qt))
                    nc.vector.tensor_copy(avS[h][:], av[:])
                # batched normalization
                recD = tl(smp, [H, L], F32, tag="recD")
                nc.vector.reciprocal(recD[:], psD[:])
                recD16 = tl(smp, [H, L], F16, tag="recD16")
                nc.vector.tensor_copy(recD16[:], recD[:])
                for h in range(H):
                    mq, rq = (h * 64) // P, (h * 64) % P
                    rep_ps = tl(psAV, [64, L], F32, tag="repps", bufs=1)
                    nc.tensor.matmul(rep_ps[:], lhsT=sel_ts[h],
                                     rhs=recD16[:], start=True,
                                     stop=True)
                    rep = tl(smp, [64, L], F16, tag="rep")
                    nc.vector.tensor_copy(rep[:], rep_ps[:])
                    nc.vector.tensor_mul(attnT[mq][rq:rq + 64, :], avS[h][:],
                                         rep[:])

            # ---- out-proj + residual (into xt -> zT)
            with tc.tile_pool(name="wp2", bufs=2) as wp2, \
                 tc.tile_pool(name="psO", bufs=2, space="PSUM") as psO:
                for mg in range(KT // 4):
                    wo = []
                    for k in range(KT):
                        t = tl(wp2, [P, 512], F16, tag=f"wo{k}")
                        nc.sync.dma_start(
                            t[:], woutT[k * P:(k + 1) * P,
                                        mg * 512:(mg + 1) * 512])
                        wo.append(t)
                    for j in range(4):
                        m = mg * 4 + j
                        ps = tl(psO, [P, L], F32, tag="ps")
                        for k in range(KT):
                            nc.tensor.matmul(ps[:],
                                             lhsT=wo[k][:, j * P:(j + 1) * P],
                                             rhs=attnT[k][:],
                                             start=(k == 0), stop=(k == KT - 1))
                        nc.vector.tensor_scalar_add(ps[:], ps[:],
                                                    bout_t[:, m:m + 1])
                        nc.vector.tensor_add(xt[m][:], ps[:], xt[m][:])  # zT

            # ---- LN1 stats + pre-LN gate logits
            # logits = rstd*(z @ (g.*gw)) - (rstd*mu)*(g@gw) + b@gw, so the
            # gate matmul runs on z directly and the logits A2A departs
            # before the y normalization loop
            for k in range(KT):
                y16.append(tl(pA, [P, L], F16, tag=f"y16_{k}"))
            with tc.tile_pool(name="lnp", bufs=3) as lnp, \
                 tc.tile_pool(name="gp", bufs=3) as gp, \
                 tc.tile_pool(name="psG", bufs=2, space="PSUM") as psG, \
                 tc.tile_pool(name="psL", bufs=1, space="PSUM") as psL:
                mu_ps = tl(psL, [1, L], F32, tag="mu")
                sq_ps = tl(psL, [1, L], F32, tag="sq")
                for k in range(KT):
                    z16 = tl(lnp, [P, L], F16, tag="z16")
                    nc.vector.tensor_copy(z16[:], xt[k][:])
                    nc.tensor.matmul(mu_ps[:], lhsT=ones16[:], rhs=z16[:],
                                     start=(k == 0), stop=(k == KT - 1))
                    sqt = tl(lnp, [P, L], F16, tag="sqt")
                    nc.scalar.activation(sqt[:], xt[k][:], ACT.Square)
                    nc.tensor.matmul(sq_ps[:], lhsT=ones16[:], rhs=sqt[:],
                                     start=(k == 0), stop=(k == KT - 1))
                gps = tl(psG, [NE, L], F32, tag="gps", bufs=1)
                for k in range(KT):
                    nc.tensor.matmul(
                        gps[:], lhsT=gw_t[:, k * NE:(k + 1) * NE],
                        rhs=xt[k][:], start=(k == 0), stop=(k == KT - 1))
                mu_r = tl(lnp, [1, L], F32, tag="mu_r")
                nc.vector.tensor_scalar_mul(mu_r[:], mu_ps[:], 1.0 / E)
                var_r = tl(lnp, [1, L], F32, tag="var_r")
                nc.vector.tensor_scalar_mul(var_r[:], sq_ps[:], 1.0 / E)
                mu2 = tl(lnp, [1, L], F32, tag="mu2")
                nc.vector.tensor_mul(mu2[:], mu_r[:], mu_r[:])
                nc.vector.tensor_sub(var_r[:], var_r[:], mu2[:])
                nc.vector.tensor_scalar_add(var_r[:], var_r[:], 1e-5)
                nc.scalar.sqrt(var_r[:], var_r[:])
                rstd_r = tl(lnp, [1, L], F32, tag="rstd_r")
                nc.vector.reciprocal(rstd_r[:], var_r[:])
                murst = tl(lnp, [1, L], F32, tag="murst")
                nc.vector.tensor_mul(murst[:], mu_r[:], rstd_r[:])
                mur16 = tl(lnp, [1, L], F16, tag="mur16")
                nc.vector.tensor_copy(mur16[:], mu_r[:])
                rstdr16 = tl(lnp, [1, L], F16, tag="rstdr16")
                nc.vector.tensor_copy(rstdr16[:], rstd_r[:])
                murst16 = tl(lnp, [1, L], F16, tag="murst16")
                nc.vector.tensor_copy(murst16[:], murst[:])
                # gate affine correction on [NE, L] (one psum bank, reused)
                rst5_ps = tl(psG, [NE, L], F32, tag="b5", bufs=1)
                nc.tensor.matmul(rst5_ps[:], lhsT=onesr16[:, 0:NE],
                                 rhs=rstdr16[:], start=True, stop=True)
                rst5 = tl(gp, [NE, L], F32, tag="rst5sb")
                nc.vector.tensor_copy(rst5[:], rst5_ps[:])
                mrst5_ps = tl(psG, [NE, L], F32, tag="b5", bufs=1)
                nc.tensor.matmul(mrst5_ps[:], lhsT=onesr16[:, 0:NE],
                                 rhs=murst16[:], start=True, stop=True)
                mrst5 = tl(gp, [NE, L], F32, tag="mrst5sb")
                nc.vector.tensor_scalar_mul(mrst5[:], mrst5_ps[:], gc1_t[:])
                lg_sb = tl(gp, [NE, L], F32, tag="lg_sb")
                nc.vector.tensor_tensor(out=lg_sb[:], in0=gps[:], in1=rst5[:],
                                        op=OP.mult)
                nc.vector.tensor_sub(lg_sb[:], lg_sb[:], mrst5[:])
                nc.vector.tensor_scalar_add(lg_sb[:], lg_sb[:], gc0_t[:])
                for ct in range(MTOK):
                    tp2 = tl(psG, [P, NE], F32, tag="tp2", bufs=1)
                    nc.tensor.transpose(tp2[:, 0:NE],
                                        lg_sb[:, ct * P:(ct + 1) * P],
                                        ident[0:NE, 0:NE])
                    lgr = tl(gp, [P, 8], F32, tag="lgr")
                    nc.vector.memset(lgr[:, NE:8], 0.0)
                    nc.vector.tensor_copy(lgr[:, 0:NE], tp2[:, 0:NE])
                    nc.sync.dma_start(sendlg[ct * P:(ct + 1) * P, :], lgr[:])

                # tiny logits AllToAll first: lets routing overlap the y A2A
                recvlg = tl(dram, [L, 8], F32, tag="recvlg")
                nc.gpsimd.collective_compute(
                    "AllToAll", OP.bypass,
                    replica_groups=[list(range(c["NC"]))],
                    ins=[sendlg[:].opt()], outs=[recvlg[:].opt()])

                # ---- y16 = LN1(z) (f16 direct; one psum bank reused)
                murep_ps = tl(psL, [P, L], F32, tag="brep", bufs=1)
                nc.tensor.matmul(murep_ps[:], lhsT=onesr16[:],
                                 rhs=mur16[:], start=True, stop=True)
                mu_rep = tl(lnp, [P, L], F32, tag="mu_rep")
                nc.vector.tensor_copy(mu_rep[:], murep_ps[:])
                rsrep_ps = tl(psL, [P, L], F32, tag="brep", bufs=1)
                nc.tensor.matmul(rsrep_ps[:], lhsT=onesr16[:],
                                 rhs=rstdr16[:], start=True, stop=True)
                rstd_rep = tl(lnp, [P, L], F32, tag="rstd_rep")
                nc.vector.tensor_copy(rstd_rep[:], rsrep_ps[:])
                for k in range(KT):
                    t1 = tl(lnp, [P, L], F32, tag="t1")
                    nc.vector.tensor_sub(t1[:], xt[k][:], mu_rep[:])
                    nc.vector.tensor_mul(t1[:], t1[:], rstd_rep[:])
                    nc.vector.tensor_scalar(
                        out=y16[k][:], in0=t1[:], scalar1=ln1g_t[:, k:k + 1],
                        scalar2=ln1b_t[:, k:k + 1], op0=OP.mult, op1=OP.add)

                # ---- y -> token-major f16 send buffer
                for ct in range(MTOK):
                    yrow = tl(gp, [P, E], F16, tag="yrow")
                    for k in range(KT):
                        tpY = tl(psG, [P, P], F16, tag="tpY")
                        nc.tensor.transpose(tpY[:], y16[k][:, ct * P:(ct + 1) * P],
                                            ident16[:])
                        nc.vector.tensor_copy(yrow[:, k * P:(k + 1) * P], tpY[:])
                    nc.sync.dma_start(send16[ct * P:(ct + 1) * P, :], yrow[:])

        # FFN weight pools open once attention SBUF is released (the
        # expert-0 half-0 prefetch in the FFN section streams during A2A)
        wf = ctx.enter_context(tc.tile_pool(name="wf", bufs=2))
        w2p = ctx.enter_context(tc.tile_pool(name="w2p", bufs=6))
        tl(w2p, [P, E], F16, tag="w2r")   # reserve: pool must not grow later

        # =========================================================
        # AllToAll + permute to group-major token order
        # =========================================================
        recv16 = tl(dram, [L, E], F16, tag="recv16")
        nc.gpsimd.collective_compute(
            "AllToAll", OP.bypass,
            replica_groups=[list(range(c["NC"]))],
            ins=[send16[:].opt()], outs=[recv16[:].opt()])
        lgbuf = tl(dram, [TOK, 8], F32, tag="lgbuf")
        nc.sync.dma_start(
            lgbuf[:].rearrange("(l i) r -> l i r", i=c["NC"]),
            recvlg[:].rearrange("(i l) r -> l i r", i=c["NC"]))

        # =========================================================
        # PHASE B: top-2 routing with capacity (overlaps the y A2A)
        # all 4 token-tiles processed as one [P, 4*NE] batch via 3D APs
        # =========================================================
        TN = NTOKT * NE
        nmat_d = tl(dram, [NTOKT, GPT, NE * CAP], F32, tag="nmat_d")

        def r3(ap):
            return ap.rearrange("p (t e) -> p t e", e=NE)

        with tc.tile_pool(name="rt", bufs=1) as rt, \
             tc.tile_pool(name="psR", bufs=1, space="PSUM") as psR:
            lg = tl(rt, [P, TN], F32, tag="lg")
            for tt in range(NTOKT):
                nc.scalar.dma_start(lg[:, tt * NE:(tt + 1) * NE],
                                    lgbuf[tt * P:(tt + 1) * P, 0:NE])
            # softmax over NE per tile (logits are small: no max-sub needed)
            ex = tl(rt, [P, TN], F32, tag="ex")
            nc.scalar.activation(ex[:], lg[:], ACT.Exp)
            sm = tl(rt, [P, NTOKT], F32, tag="sm")
            nc.vector.reduce_sum(sm[:], r3(ex[:]), axis=AX.X)
            rcp = tl(rt, [P, NTOKT], F32, tag="rcp")
            nc.vector.reciprocal(rcp[:], sm[:])
            raw = tl(rt, [P, TN], F32, tag="raw")
            nc.vector.tensor_tensor(
                out=r3(raw[:]), in0=r3(ex[:]),
                in1=rcp[:].unsqueeze(2).to_broadcast([P, NTOKT, NE]),
                op=OP.mult)

            def top1(rawt, tag):
                g = tl(rt, [P, NTOKT], F32, tag=f"g{tag}")
                nc.vector.reduce_max(g[:], r3(rawt), axis=AX.X)
                eq = tl(rt, [P, TN], F32, tag=f"eq{tag}")
                nc.vector.tensor_tensor(
                    out=r3(eq[:]), in0=r3(rawt),
                    in1=g[:].unsqueeze(2).to_broadcast([P, NTOKT, NE]),
                    op=OP.is_ge)
                cs = tl(rt, [P, TN], F32, tag=f"cs{tag}")
                nc.vector.memset(r3(cs[:])[:, :, 0:1], 0.0)
                for j in range(1, NE):
                    nc.vector.tensor_add(r3(cs[:])[:, :, j:j + 1],
                                         r3(cs[:])[:, :, j - 1:j],
                                         r3(eq[:])[:, :, j - 1:j])
                fst = tl(rt, [P, TN], F32, tag=f"fst{tag}")
                nc.vector.tensor_scalar(out=fst[:], in0=cs[:], scalar1=0.5,
                                        scalar2=None, op0=OP.is_lt)
                m_ = tl(rt, [P, TN], F32, tag=f"m{tag}")
                nc.vector.tensor_mul(m_[:], eq[:], fst[:])
                return g, m_

            g1, m1r = top1(raw[:], "1")
            raw2 = tl(rt, [P, TN], F32, tag="raw2")
            nc.vector.tensor_mul(raw2[:], raw[:], m1r[:])
            nc.vector.tensor_sub(raw2[:], raw[:], raw2[:])
            g2, m2r = top1(raw2[:], "2")
            den = tl(rt, [P, NTOKT], F32, tag="den")
            nc.vector.tensor_add(den[:], g1[:], g2[:])
            nc.vector.tensor_scalar_add(den[:], den[:], 1e-9)
            rd = tl(rt, [P, NTOKT], F32, tag="rd")
            nc.vector.reciprocal(rd[:], den[:])
            g1n = tl(rt, [P, NTOKT], F32, tag="g1n")
            nc.vector.tensor_mul(g1n[:], g1[:], rd[:])
            g2n = tl(rt, [P, NTOKT], F32, tag="g2n")
            nc.vector.tensor_mul(g2n[:], g2[:], rd[:])

            # capacity by position within group (cumsum over tokens = tri/ob
            # matmuls; batched over all 4 tiles)
            pos1 = tl(psR, [P, TN], F32, tag="pos1")
            nc.tensor.matmul(pos1[:], lhsT=tri_t[:], rhs=m1r[:],
                             start=True, stop=True)
            keep1 = tl(rt, [P, TN], F32, tag="keep1")
            nc.vector.tensor_scalar(out=keep1[:], in0=pos1[:],
                                    scalar1=CAP - 0.5, scalar2=None,
                                    op0=OP.is_lt)
            m1 = tl(rt, [P, TN], F32, tag="m1k")
            nc.vector.tensor_mul(m1[:], m1r[:], keep1[:])
            pos2 = tl(psR, [P, TN], F32, tag="pos2")
            nc.tensor.matmul(pos2[:], lhsT=tri_t[:], rhs=m2r[:],
                             start=True, stop=False)
            nc.tensor.matmul(pos2[:], lhsT=ob_t[:], rhs=m1[:],
                             start=False, stop=True)
            keep2 = tl(rt, [P, TN], F32, tag="keep2")
            nc.vector.tensor_scalar(out=keep2[:], in0=pos2[:],
                                    scalar1=CAP - 0.5, scalar2=None,
                                    op0=OP.is_lt)
            m2 = tl(rt, [P, TN], F32, tag="m2k")
            nc.vector.tensor_mul(m2[:], m2r[:], keep2[:])

            def dotE(a_ap, b_ap, tag):
                t5 = tl(rt, [P, TN], F32, tag=f"t5{tag}")
                nc.vector.tensor_mul(t5[:], a_ap, b_ap)
                o = tl(rt, [P, NTOKT], F32, tag=f"o{tag}")
                nc.vector.reduce_sum(o[:], r3(t5[:]), axis=AX.X)
                return o

            m1f = tl(rt, [P, NTOKT], F32, tag="m1f")
            nc.vector.reduce_sum(m1f[:], r3(m1[:]), axis=AX.X)
            m2f = tl(rt, [P, NTOKT], F32, tag="m2f")
            nc.vector.reduce_sum(m2f[:], r3(m2[:]), axis=AX.X)
            nc.vector.tensor_mul(gca[:], g1n[:], m1f[:])
            nc.vector.tensor_mul(gcb[:], g2n[:], m2f[:])
            p1 = dotE(pos1[:], m1[:], "p1")
            p2 = dotE(pos2[:], m2[:], "p2")
            e1 = dotE(iotae_t[:], m1[:], "e1")
            e2 = dotE(iotae_t[:], m2[:], "e2")
            # capacity-dropped ranks alias to expert 0 (sums of zeroed masks);
            # bump them to a sentinel so they match no expert in m1e/m2e
            sent = tl(rt, [P, NTOKT], F32, tag="sent")
            nc.vector.tensor_scalar(out=sent[:], in0=m1f[:], scalar1=-64.0,
                                    scalar2=64.0, op0=OP.mult, op1=OP.add)
            nc.vector.tensor_add(e1[:], e1[:], sent[:])
            nc.vector.tensor_scalar(out=sent[:], in0=m2f[:], scalar1=-64.0,
                                    scalar2=64.0, op0=OP.mult, op1=OP.add)
            nc.vector.tensor_add(e2[:], e2[:], sent[:])

            # per-(expert, tile) gather indices + gate weights for the
            # streaming combine: idx = e*GCAP + group*CAP + pos if the token
            # routed to e (rank 1 or 2), else the shared zero row
            ZROW = float(NE * GCAP)
            lidx1 = tl(rt, [P, NTOKT], F32, tag="lidx1")
            nc.vector.tensor_add(lidx1[:], p1[:], gb2_t[:])
            lidx2 = tl(rt, [P, NTOKT], F32, tag="lidx2")
            nc.vector.tensor_add(lidx2[:], p2[:], gb2_t[:])
            m1e = tl(rt, [P, TN], F32, tag="m1e")
            nc.vector.tensor_tensor(
                out=r3(m1e[:]), in0=e1[:].unsqueeze(2).to_broadcast([P, NTOKT, NE]),
                in1=r3(iotae_t[:]), op=OP.is_equal)
            m2e = tl(rt, [P, TN], F32, tag="m2e")
            nc.vector.tensor_tensor(
                out=r3(m2e[:]), in0=e2[:].unsqueeze(2).to_broadcast([P, NTOKT, NE]),
                in1=r3(iotae_t[:]), op=OP.is_equal)
            ga_ = tl(rt, [P, TN], F32, tag="ga_")
            nc.vector.tensor_tensor(
                out=r3(ga_[:]), in0=gca[:].unsqueeze(2).to_broadcast([P, NTOKT, NE]),
                in1=r3(m1e[:]), op=OP.mult)
            gb_ = tl(rt, [P, TN], F32, tag="gb_")
            nc.vector.tensor_tensor(
                out=r3(gb_[:]), in0=gcb[:].unsqueeze(2).to_broadcast([P, NTOKT, NE]),
                in1=r3(m2e[:]), op=OP.mult)
            nc.vector.tensor_add(gsel[:], ga_[:], gb_[:])
            ia_ = tl(rt, [P, TN], F32, tag="ia_")
            nc.vector.tensor_tensor(
                out=r3(ia_[:]), in0=lidx1[:].unsqueeze(2).to_broadcast([P, NTOKT, NE]),
                in1=r3(m1e[:]), op=OP.mult)
            ib_ = tl(rt, [P, TN], F32, tag="ib_")
            nc.vector.tensor_tensor(
                out=r3(ib_[:]), in0=lidx2[:].unsqueeze(2).to_broadcast([P, NTOKT, NE]),
                in1=r3(m2e[:]), op=OP.mult)
            # idx = (lidx1+e*G)*m1e + (lidx2+e*G)*m2e + ZROW*(1-m1e-m2e)
            # built as: (lidx1*m1e + lidx2*m2e) + e*G*(m1e+m2e) + ZROW*(1-..)
            zm = tl(rt, [P, TN], F32, tag="zm")
            nc.vector.tensor_add(zm[:], m1e[:], m2e[:])
            idxf = tl(rt, [P, TN], F32, tag="idxf")
            nc.vector.tensor_add(idxf[:], ia_[:], ib_[:])
            eg_ = tl(rt, [P, TN], F32, tag="eg_")
            nc.vector.tensor_scalar_add(eg_[:], egcap_t[:], -ZROW)
            nc.vector.tensor_mul(eg_[:], eg_[:], zm[:])
            nc.vector.tensor_add(idxf[:], idxf[:], eg_[:])
            nc.vector.tensor_scalar_add(idxf[:], idxf[:], ZROW)
            for e in range(NE):
                for tt in range(NTOKT):
                    nc.vector.tensor_copy(idxsel[(e, tt)][:],
                                          r3(idxf[:])[:, tt, e:e + 1])

            # slot -> source-token matrix, batched over tiles
            oh1 = tl(rt, [P, NTOKT * CAP], F32, tag="oh1")
            nc.vector.tensor_tensor(
                out=oh1[:].rearrange("p (t c) -> p t c", c=CAP),
                in0=p1[:].unsqueeze(2).to_broadcast([P, NTOKT, CAP]),
                in1=iotac_t[:].rearrange("p (t c) -> p t c", c=CAP),
                op=OP.is_equal)
            oh2 = tl(rt, [P, NTOKT * CAP], F32, tag="oh2")
            nc.vector.tensor_tensor(
                out=oh2[:].rearrange("p (t c) -> p t c", c=CAP),
                in0=p2[:].unsqueeze(2).to_broadcast([P, NTOKT, CAP]),
                in1=iotac_t[:].rearrange("p (t c) -> p t c", c=CAP),
                op=OP.is_equal)
            D = tl(rt, [P, NTOKT * NE * CAP], F32, tag="D")
            nc.vector.tensor_tensor(
                out=D[:].rearrange("p (t e c) -> p t e c", e=NE, c=CAP),
                in0=r3(m1[:]).unsqueeze(3).to_broadcast([P, NTOKT, NE, CAP]),
                in1=oh1[:].rearrange("p (t c) -> p t c", c=CAP)
                    .unsqueeze(2).to_broadcast([P, NTOKT, NE, CAP]),
                op=OP.mult)
            D2 = tl(rt, [P, NTOKT * NE * CAP], F32, tag="D2")
            nc.vector.tensor_tensor(
                out=D2[:].rearrange("p (t e c) -> p t e c", e=NE, c=CAP),
                in0=r3(m2[:]).unsqueeze(3).to_broadcast([P, NTOKT, NE, CAP]),
                in1=oh2[:].rearrange("p (t c) -> p t c", c=CAP)
                    .unsqueeze(2).to_broadcast([P, NTOKT, NE, CAP]),
                op=OP.mult)
            nc.vector.tensor_add(D[:], D[:], D2[:])
            nm = tl(psR, [GPT, NTOKT * NE * CAP], F32, tag="nm")
            nc.tensor.matmul(nm[:], lhsT=nsel_t[:], rhs=D[:],
                             start=True, stop=True)
            nm_sb = tl(rt, [GPT, NTOKT * NE * CAP], F32, tag="nm_sb")
            nc.vector.tensor_copy(nm_sb[:], nm[:])
            nc.sync.dma_start(
                nmat_d[:].rearrange("t g x -> g t x"),
                nm_sb[:].rearrange("g (t x) -> g t x", x=NE * CAP))

        # slot source-row indices (per expert / slot-tile), via DRAM relayout
        with tc.tile_pool(name="ip", bufs=2) as ip:
            for e in range(NE):
                for st in range(nslt):
                    f_ = tl(ip, [spt, 1], F32, tag="f")
                    src = nmat_d[:][st * tpst:(st + 1) * tpst, :,
                                    e * CAP:(e + 1) * CAP]
                    nc.sync.dma_start(f_[:], src)
                    nc.vector.tensor_scalar_add(f_[:], f_[:], gbase_ts[st][:])
                    nc.vector.tensor_copy(islot[e][st][:], f_[:])

        # y A2A result -> group-major token order (issued here so the sync
        # queue is not blocked behind the big A2A during routing)
        ybuf16 = tl(dram, [TOK, E], F16, tag="ybuf16")
        nc.sync.dma_start(
            ybuf16[:].rearrange("(l i) r -> l i r", i=c["NC"]),
            recv16[:].rearrange("(i l) r -> l i r", i=c["NC"]))

        # =========================================================
        # expert FFN (fused w1/w2 per expert) + streaming combine:
        # each expert's output is gathered and accumulated into acc[tt]
        # while the next expert computes, so only the last expert's
        # combine + LN2 remain after the FFN
        # =========================================================
        eobuf = tl(dram, [NE * GCAP + 1, E], F16, tag="eobuf")
        G1 = 2
        NMG = 32 // G1
        nc.gpsimd.dma_start(ln2g_sb[:], ln2g)
        nc.gpsimd.dma_start(ln2b_sb[:], ln2b)
        with tc.tile_pool(name="einp", bufs=1) as einp, \
             tc.tile_pool(name="eintp", bufs=1) as eintp, \
             tc.tile_pool(name="htp", bufs=2) as htp, \
             tc.tile_pool(name="eop", bufs=2) as eop, \
             tc.tile_pool(name="cmb", bufs=2) as cmb, \
             tc.tile_pool(name="psF", bufs=1, space="PSUM") as psF, \
             tc.tile_pool(name="psW2", bufs=1, space="PSUM") as psW2, \
             tc.tile_pool(name="psT", bufs=2, space="PSUM") as psT:
            # reserve pass: touch every tag once so no pool grows after a
            # later pool has stacked above it (late growth deadlocks)
            for e_ in range(NE):
                for st in range(nslt):
                    tl(einp, [spt, E], F16, tag=f"g{e_}_{st}")
            for e_ in range(NE):
                for k_ in range(KT):
                    tl(eintp, [P, GCAP], F16, tag=f"einT{e_}_{k_}")
            for kh_ in range(HIDT):
                tl(htp, [P, GCAP], F16, tag=f"ht{kh_}")
            tl(eop, [P, 512], F16, tag="eo16")
            tl(cmb, [1, E], F16, tag="zr")
            tl(cmb, [P, E], F16, tag="ysb")
            tl(cmb, [P, E], F16, tag="og")
            tl(cmb, [P, E], F32, tag="sg")
            # zero row for non-routed gathers
            zr = tl(cmb, [1, E], F16, tag="zr")
            nc.vector.memset(zr[:], 0.0)
            nc.sync.dma_start(eobuf[NE * GCAP:NE * GCAP + 1, :], zr[:])
            # acc[tt] starts as the residual y
            for tt in range(NTOKT):
                ysb = tl(cmb, [P, E], F16, tag="ysb")
                nc.sync.dma_start(ysb[:], ybuf16[tt * P:(tt + 1) * P, :])
                nc.scalar.copy(acc[tt][:], ysb[:])
            # prefetch ALL expert gathers up front (gpsimd queue)
            eins = {}
            for e in range(NE):
                for st in range(nslt):
                    g_ = tl(einp, [spt, E], F16, tag=f"g{e}_{st}")
                    nc.gpsimd.indirect_dma_start(
                        out=g_[:], out_offset=None, in_=ybuf16[:],
                        in_offset=bass.IndirectOffsetOnAxis(
                            ap=islot[e][st][:, :1], axis=0))
                    eins[(e, st)] = g_

            # w1 weight tiles: half0 of expert 0 up front; thereafter each
            # (expert, half) trickles in one [128,2048] tile per mg group of
            # the previous half, so the stream never bursts
            halves = [(e, hf) for e in range(NE) for hf in range(2)]
            w1trickle = {"next": {}}
            w1cur = {}
            for k in range(KT):
                t = tl(wf, [P, 2048], F16, tag=f"w1_{k}")
                nc.sync.dma_start(t[:], w1[0][k * P:(k + 1) * P, 0:2048])
                w1cur[k] = t

            for e in range(NE):
                # einT for this expert (just-in-time on tensor queue)
                einT = {}
                for k in range(KT):
                    t_ = tl(eintp, [P, GCAP], F16, tag=f"einT{e}_{k}")
                    for st in range(nslt):
                        tp3 = tl(psT, [P, P], F16, tag="tp3")
                        nc.tensor.transpose(tp3[:, 0:spt],
                                            eins[(e, st)][:, k * P:(k + 1) * P],
                                            ident16[0:spt, 0:spt])
                        nc.vector.tensor_copy(t_[:, st * P:st * P + spt],
                                              tp3[:, 0:spt])
                    einT[k] = t_
                hts = {}
                pw = [tl(psW2, [P, 512], F32, tag=f"pw{i}")
                      for i in range(2 * nslt)]

                def w2_block(kh):
                    w2r = tl(w2p, [P, E], F16, tag="w2r")
                    wdma(w2r[:], w2[e][kh * P:(kh + 1) * P, :])
                    for sb in range(nslt):
                        for ch in range(2):
                            nc.tensor.matmul(
                                pw[sb * 2 + ch][:],
                                lhsT=hts[kh][:, sb * P:sb * P + spt],
                                rhs=w2r[:, ch * 512:(ch + 1) * 512],
                                start=(kh == 0), stop=(kh == HIDT - 1))

                for mg in range(NMG):
                    half, hmg = mg // 8, mg % 8
                    hseq = e * 2 + half
                    if hmg == 0:
                        if hseq > 0:
                            w1cur = w1trickle["next"]
                        w1trickle["next"] = {}
                    # trickle-prefetch the NEXT half two tiles per mg over
                    # the first four groups (no burst, ample lead time)
                    if hseq + 1 < len(halves) and hmg < 4:
                        en, hn = halves[hseq + 1]
                        for kpre in (2 * hmg, 2 * hmg + 1):
                            t = tl(wf, [P, 2048], F16, tag=f"w1_{kpre}")
                            wdma(t[:], w1[en][kpre * P:(kpre + 1) * P,
                                              hn * 2048:(hn + 1) * 2048])
                            w1trickle["next"][kpre] = t
                    pss = [tl(psF, [P, GCAP], F32, tag=f"ps{j}")
                           for j in range(G1)]
                    for k in range(KT):
                        for j in range(G1):
                            col = (hmg * G1 + j) * P
                            nc.tensor.matmul(
                                pss[j][:], lhsT=w1cur[k][:, col:col + P],
                                rhs=einT[k][:], start=(k == 0),
                                stop=(k == KT - 1))
                    for j in range(G1):
                        kh = mg * G1 + j
                        ht_ = tl(htp, [P, GCAP], F16, tag=f"ht{kh}")
                        nc.scalar.activation(ht_[:], pss[j][:], ACT.Gelu)
                        hts[kh] = ht_
                    if mg > 0:
                        for j in range(G1):
                            w2_block((mg - 1) * G1 + j)
                for j in range(G1):
                    w2_block((NMG - 1) * G1 + j)
                for sb in range(nslt):
                    for ch in range(2):
                        eo16 = tl(eop, [P, 512], F16, tag="eo16")
                        nc.vector.tensor_copy(eo16[0:spt, :],
                                              pw[sb * 2 + ch][0:spt, :])
                        nc.sync.dma_start(
                            eobuf[e * GCAP + sb * P:e * GCAP + sb * P + spt,
                                  ch * 512:(ch + 1) * 512], eo16[0:spt, :])

                # streaming combine for this expert (overlaps next expert)
                for tt in range(NTOKT):
                    og = tl(cmb, [P, E], F16, tag="og")
                    nc.gpsimd.indirect_dma_start(
                        out=og[:], out_offset=None, in_=eobuf[:],
                        in_offset=bass.IndirectOffsetOnAxis(
                            ap=idxsel[(e, tt)][:, :1], axis=0))
                    sg = tl(cmb, [P, E], F32, tag="sg")
                    nc.scalar.activation(
                        sg[:], og[:], ACT.Copy,
                        scale=gsel[:, tt * NE + e:tt * NE + e + 1])
                    nc.vector.tensor_add(acc[tt][:], acc[tt][:], sg[:])

        # =========================================================
        # LN2 -> out
        # =========================================================
        with tc.tile_pool(name="cb", bufs=2) as cb:
            for tt in range(NTOKT):
                z = acc[tt]
                mu = tl(cb, [P, 1], F32, tag="mu")
                nc.vector.reduce_sum(mu[:], z[:], axis=AX.X)
                nc.vector.tensor_scalar_mul(mu[:], mu[:], 1.0 / E)
                xc = tl(cb, [P, E], F32, tag="xc")
                nc.vector.tensor_scalar(out=xc[:], in0=z[:], scalar1=mu[:],
                                        scalar2=None, op0=OP.subtract)
                scr = tl(cb, [P, E], F32, tag="scr")
                ssq = tl(cb, [P, 1], F32, tag="ssq")
                nc.scalar.activation(scr[:], xc[:], ACT.Square, accum_out=ssq[:])
                nc.vector.tensor_scalar(out=ssq[:], in0=ssq[:], scalar1=1.0 / E,
                                        scalar2=1e-5, op0=OP.mult, op1=OP.add)
                nc.scalar.sqrt(ssq[:], ssq[:])
                rstd = tl(cb, [P, 1], F32, tag="rstd")
                nc.vector.reciprocal(rstd[:], ssq[:])
                nc.vector.tensor_scalar_mul(xc[:], xc[:], rstd[:])
                yo = tl(cb, [P, E], F32, tag="yo")
                nc.vector.tensor_mul(yo[:], xc[:], ln2g_sb[:])
                nc.vector.tensor_add(yo[:], yo[:], ln2b_sb[:])
                nc.sync.dma_start(out[tt * P:(tt + 1) * P, :], yo[:])

    nc.compile()
    return nc


# =========================================================
# host side
# =========================================================
_CACHE = {}


def host_prep(cfg, inputs):
    """Full (unsharded) inputs -> list of per-core input maps."""
    E = cfg["E"]
    x = np.asarray(inputs["x"], np.float32)
    t = np.asarray(inputs["time"], np.float32)
    shared = dict(
        wqkvT=np.ascontiguousarray(
            np.asarray(inputs["w_qkv"], np.float32).T.astype(np.float16)),
        bqk=np.ascontiguousarray(
            np.asarray(inputs["b_qkv"], np.float32)[:2 * E, None]),
        bvrep=np.ascontiguousarray(
            np.tile(np.asarray(inputs["b_qkv"], np.float32)[None, 2 * E:], (P, 1))),
        woutT=np.ascontiguousarray(
            np.asarray(inputs["w_out"], np.float32).T.astype(np.float16)),
        bout=np.ascontiguousarray(np.asarray(inputs["b_out"], np.float32)[:, None]),
        ln1g=np.ascontiguousarray(np.asarray(inputs["ln1_g"], np.float32)[:, None]),
        ln1b=np.ascontiguousarray(np.asarray(inputs["ln1_b"], np.float32)[:, None]),
        ln2grep=np.ascontiguousarray(
            np.tile(np.asarray(inputs["ln2_g"], np.float32)[None, :], (P, 1))),
        ln2brep=np.ascontiguousarray(
            np.tile(np.asarray(inputs["ln2_b"], np.float32)[None, :], (P, 1))),
        gatew=np.ascontiguousarray(
            np.asarray(inputs["ln1_g"], np.float32)[:, None]
            * np.asarray(inputs["gate_w"], np.float32)),
        gatec0=np.ascontiguousarray(
            (np.asarray(inputs["ln1_b"], np.float32)
             @ np.asarray(inputs["gate_w"], np.float32))[:, None]),
        gatec1=np.ascontiguousarray(
            (np.asarray(inputs["ln1_g"], np.float32)
             @ np.asarray(inputs["gate_w"], np.float32))[:, None]),
        w1=np.ascontiguousarray(np.asarray(inputs["w1"]).astype(np.float16)),
        w2=np.ascontiguousarray(np.asarray(inputs["w2"]).astype(np.float16)),
    )
    in_maps = []
    for cid in range(cfg["NC"]):
        m = dict(shared)
        m["xT"] = np.ascontiguousarray(x[:, cid, :].T)
        m["tcol"] = np.ascontiguousarray(t[:, cid][:, None])
        m["trep"] = np.ascontiguousarray(np.tile(t[:, cid][None, :], (P, 1)))
        in_maps.append(m)
    return in_maps


def assemble(cfg, results):
    """Per-core 'out' (TOK, E) -> full (L, B, E)."""
    L, B, E, LC = cfg["L"], cfg["B"], cfg["E"], cfg["LC"]
    full = np.empty((L, B, E), np.float32)
    for cid in range(cfg["NC"]):
        o = np.asarray(results[cid]["out"]).reshape(LC, B, E)
        full[cid * LC:(cid + 1) * LC, :, :] = o
    return full


def get_built():
    if "full" not in _CACHE:
        cfg = make_cfg(FULL)
        _CACHE["full"] = (build_bass(cfg), cfg)
    return _CACHE["full"]


def kernel(**inputs):
    nc, cfg = get_built()
    in_maps = host_prep(cfg, inputs)
    res = run_bass_kernel_spmd(nc, in_maps, core_ids=list(range(cfg["NC"])))
    return assemble(cfg, res.results)
